# revision 3
# baseline (speedup 1.0000x reference)
"""Mimi-style GQA attention (RoPE + 250-wide sliding causal window) on 8 TRN2 NeuronCores.

Sharding: core c handles batch b=c//4 and KV-head group g=c%4 (4 query heads +
1 KV head). Wq/Wk/Wv column-sharded, Wo row-sharded along the head dim; each
core emits a partial [S, D] f16 output; host sums the 4 partials per batch.

Strip-streamed pipeline (4 q-strips of 512, software-pipelined over 6
iterations) to keep the PE matmul queue dense (HAM stays un-throttled):
  iter i: proj+rope strip i | scores for strip i-1's k-tiles | PV+norm for
  q-strip i-1 | output projection for strip i-2.
Key points vs the phase-barrier version:
  - hs streamed per strip; projections start as soon as strip 0 lands.
  - V projected directly in [pos, hd] orientation (hs chunk stationary), no
    PE transposes; ones column in vaug gives softmax denominators for free.
  - PV accumulation needs no zero-seed matmuls: pieces ascend in kt so the
    first piece (start=True) clears the bank and later start=False matmuls
    accumulate where has_written is set and overwrite fresh columns.
  - Softmax normalization: ACT row-copy of denominators -> DMA reshape to
    [8,128] -> DVE reciprocal_approx_fast -> DMA -> gpsimd partition
    broadcast -> DVE multiplies; pipelined one iteration ahead of the O-proj.
  - PSUM = exactly 8 banks: 4 rotating [128,512] matmul banks + 2x2 PV banks.
"""
import os
import sys

for _p in ("/opt/trn_rl_repo", "/root/.axon_site/_ro/trn_rl_repo"):
    if os.path.isdir(_p) and _p not in sys.path:
        sys.path.append(_p)

import numpy as np
import concourse.bass as bass
import concourse.mybir as mybir
import concourse.tile as tile
from concourse import bacc
from concourse.bass_utils import run_bass_kernel_spmd

F32 = mybir.dt.float32
F16 = mybir.dt.float16
AF = mybir.ActivationFunctionType
OP = mybir.AluOpType

B, S, D = 2, 2048, 1024
H, HK, HD = 16, 4, 64
WINDOW = 250
SCALE = 1.0 / np.sqrt(HD)
THETA = 10000.0
NKT = S // 128          # 16 k-tiles
NST = S // 512          # 4 q-strips
WIN = 384               # padded per-k-tile q-window


def _pv_pieces(s):
    """PV pieces for q-strip s, ascending kt: list of (kt, c0, c1) ranges."""
    out = []
    for kt in range(max(0, 4 * s - 2), min(NKT - 1, 4 * s + 3) + 1):
        j0 = 128 * kt
        w = min(WIN, S - j0)
        c_lo = max(0, 512 * s - j0)
        c_hi = min(w, 512 * (s + 1) - j0)
        if c_lo < c_hi:
            out.append((kt, c_lo, c_hi))
    return out


def _build():
    nc = bacc.Bacc(None, target_bir_lowering=False)

    hsT4 = nc.declare_dram_parameter("hsT4", [NST, 8, 128, 512], F16, isOutput=False)
    wq = nc.declare_dram_parameter("wqT", [8, 128, 256], F16, isOutput=False)
    wk = nc.declare_dram_parameter("wkT", [8, 128, 64], F16, isOutput=False)
    wv = nc.declare_dram_parameter("wvT", [8, 128, 64], F16, isOutput=False)
    wo = nc.declare_dram_parameter("woT", [2, 128, D], F16, isOutput=False)
    cosd = nc.declare_dram_parameter("cos2", [128, S], F16, isOutput=False)
    sind = nc.declare_dram_parameter("sinS2", [128, S], F16, isOutput=False)
    maskd = nc.declare_dram_parameter("bandmask", [128, WIN], F16, isOutput=False)
    permd = nc.declare_dram_parameter("permT", [128, 128], F16, isOutput=False)
    o_part = nc.declare_dram_parameter("o_part", [S, D], F16, isOutput=True)

    with tile.TileContext(nc) as tc:
        with (
            tc.tile_pool(name="persist", bufs=1) as pp,
            tc.tile_pool(name="hts", bufs=4) as hpool,
            tc.tile_pool(name="work", bufs=2) as wkp,
            tc.tile_pool(name="expm", bufs=36) as ep,
            tc.tile_pool(name="norm", bufs=3) as npool,
            tc.tile_pool(name="ost", bufs=3) as opool,
            tc.tile_pool(name="pmm", bufs=4, space="PSUM") as pmm,
            tc.tile_pool(name="ppv", bufs=2, space="PSUM") as ppv,
        ):
            # ---- persistent SBUF tiles ----
            warm = pp.tile([128, 512], F16, tag="warm")
            nc.vector.memset(warm, 0.0)

            wq_sb = pp.tile([128, 8, 256], F16, tag="wq")
            wk_sb = pp.tile([128, 8, 64], F16, tag="wk")
            wv_sb = pp.tile([128, 8, 64], F16, tag="wv")
            wo_sb = pp.tile([128, 2, D], F16, tag="wo")
            cos_sb = pp.tile([128, S], F16, tag="cos")
            sin_sb = pp.tile([128, S], F16, tag="sin")
            mask_sb = pp.tile([128, WIN], F16, tag="mask")
            perm_sb = pp.tile([128, 128], F16, tag="perm")

            qT = [pp.tile([128, S], F16, tag=f"qT{m}", name=f"qT{m}") for m in range(2)]
            kdup = pp.tile([128, S], F16, tag="kdup")
            vaug = pp.tile([128, NKT, 65], F16, tag="vaug")
            nc.vector.memset(vaug[:, :, 64:65], 1.0)
            aT = [pp.tile([128, S], F16, tag=f"aT{m}", name=f"aT{m}") for m in range(2)]

            # ---- input DMA stream (ordered on the sync queue) ----
            nc.sync.dma_start(out=wq_sb, in_=wq.rearrange("a p c -> p a c"))
            nc.sync.dma_start(out=wk_sb, in_=wk.rearrange("a p c -> p a c"))
            nc.sync.dma_start(out=wv_sb, in_=wv.rearrange("a p c -> p a c"))
            nc.sync.dma_start(out=perm_sb, in_=permd[:, :])
            nc.sync.dma_start(out=mask_sb, in_=maskd[:, :])
            ht = []
            for s_ in range(NST):
                t = hpool.tile([128, 8, 512], F16, tag="ht", name=f"ht{s_}")
                nc.sync.dma_start(out=t, in_=hsT4[s_].rearrange("a p c -> p a c"))
                ht.append(t)
                if s_ == 0:
                    nc.sync.dma_start(out=cos_sb, in_=cosd[:, :])
                    nc.sync.dma_start(out=sin_sb, in_=sind[:, :])
                if s_ == 1:
                    nc.sync.dma_start(out=wo_sb, in_=wo.rearrange("a p c -> p a c"))

            # ---- PE warm-up (no DMA deps, ramps HAM while inputs stream) ----
            for _w in range(12):
                wmm = pmm.tile([128, 512], F32, tag="mm", name=f"warm{_w}")
                nc.tensor.matmul(wmm, warm[:, 0:128], warm, start=True, stop=True)

            expm = {}

            def proj_strip(s):
                sl = bass.ts(s, 512)
                raws = []
                # q projections (m = head-pair half of the moving weights)
                for m in range(2):
                    qps = pmm.tile([128, 512], F32, tag="mm", name=f"q{s}_{m}")
                    for dt in range(8):
                        nc.tensor.matmul(qps, wq_sb[:, dt, bass.ts(m, 128)],
                                         ht[s][:, dt, :],
                                         start=(dt == 0), stop=(dt == 7))
                    raw = wkp.tile([128, 512], F16, tag=f"raw{m}")
                    nc.scalar.copy(raw, qps)
                    raws.append(raw)
                # k projection (64 hd rows)
                kps = pmm.tile([128, 512], F32, tag="mm", name=f"k{s}")
                for dt in range(8):
                    nc.tensor.matmul(kps[0:64, :], wk_sb[:, dt, :], ht[s][:, dt, :],
                                     start=(dt == 0), stop=(dt == 7))
                rawk = wkp.tile([64, 512], F16, tag="rawk")
                nc.scalar.copy(rawk, kps[0:64, :])
                # v projection, direct [pos, hd] orientation (hs chunk stationary)
                vps = pmm.tile([128, 512], F32, tag="mm", name=f"v{s}")
                for pt in range(4):
                    for dt in range(8):
                        nc.tensor.matmul(vps[:, bass.ts(pt, 64)],
                                         ht[s][:, dt, bass.ts(pt, 128)],
                                         wv_sb[:, dt, :],
                                         start=(pt == 0 and dt == 0),
                                         stop=(pt == 3 and dt == 7),
                                         skip_group_check=True)
                nc.vector.tensor_copy(vaug[:, 4 * s:4 * s + 4, 0:64], vps[:, 0:256])
                # rope rotations + combines
                for m in range(2):
                    rot = pmm.tile([128, 512], F32, tag="mm", name=f"rot{s}_{m}")
                    nc.tensor.matmul(rot, perm_sb, raws[m], start=True, stop=True)
                    t1 = wkp.tile([128, 512], F16, tag=f"t1{m}")
                    nc.vector.tensor_tensor(out=t1, in0=rot, in1=sin_sb[:, sl],
                                            op=OP.mult)
                    t2 = wkp.tile([128, 512], F16, tag=f"t2{m}")
                    nc.gpsimd.tensor_tensor(out=t2, in0=raws[m], in1=cos_sb[:, sl],
                                            op=OP.mult)
                    nc.vector.tensor_tensor(out=qT[m][:, sl], in0=t1, in1=t2,
                                            op=OP.add)
                rotk = pmm.tile([128, 512], F32, tag="mm", name=f"rotk{s}")
                nc.tensor.matmul(rotk[0:64, :], perm_sb[0:64, 0:64], rawk,
                                 start=True, stop=True)
                t1k = wkp.tile([64, 512], F16, tag="t1k")
                nc.vector.tensor_tensor(out=t1k, in0=rotk[0:64, :],
                                        in1=sin_sb[0:64, sl], op=OP.mult)
                t2k = wkp.tile([64, 512], F16, tag="t2k")
                nc.gpsimd.tensor_tensor(out=t2k, in0=rawk, in1=cos_sb[0:64, sl],
                                        op=OP.mult)
                nc.vector.tensor_tensor(out=kdup[0:64, sl], in0=t1k, in1=t2k,
                                        op=OP.add)
                nc.sync.dma_start(out=kdup[64:128, sl], in_=kdup[0:64, sl])

            def scores_kts(s):
                """Scores + exp + band-mask for the 4 k-tiles of strip s."""
                for kt in range(4 * s, 4 * s + 4):
                    j0 = 128 * kt
                    w = min(WIN, S - j0)
                    for h in range(4):
                        m, half = h // 2, (h % 2) * 64
                        pss = pmm.tile([128, 512], F32, tag="mm",
                                       name=f"sc{kt}_{h}")
                        nc.tensor.matmul(
                            pss[:, 0:w],
                            kdup[half:half + 64, bass.ts(kt, 128)],
                            qT[m][half:half + 64, j0:j0 + w],
                            start=True, stop=True)
                        et = ep.tile([128, WIN], F16, tag="e")
                        nc.scalar.activation(et[:, 0:w], pss[:, 0:w], AF.Exp,
                                             scale=float(SCALE))
                        if h < 3:
                            nc.vector.tensor_tensor(out=et[:, 0:w], in0=et[:, 0:w],
                                                    in1=mask_sb[:, 0:w], op=OP.mult)
                        else:
                            nc.gpsimd.tensor_tensor(out=et[:, 0:w], in0=et[:, 0:w],
                                                    in1=mask_sb[:, 0:w], op=OP.mult)
                        expm[(kt, h)] = et

            def pv_norm(s):
                pieces = _pv_pieces(s)
                sl = bass.ts(s, 512)
                for hp in range(2):
                    pvp = ppv.tile([65, 2, 512], F32, tag="pv", name=f"pv{s}_{hp}")
                    for i, (kt, c0, c1) in enumerate(pieces):
                        base = 128 * kt + c0 - 512 * s
                        for hh in range(2):
                            nc.tensor.matmul(pvp[:, hh, base:base + (c1 - c0)],
                                             vaug[:, kt, 0:65],
                                             expm[(kt, 2 * hp + hh)][:, c0:c1],
                                             start=(i == 0), stop=(i == len(pieces) - 1),
                                             skip_group_check=True)
                    # softmax denominators live in row 64 (ones column of vaug)
                    srow = npool.tile([1, 2, 512], F32, tag="srow")
                    nc.scalar.copy(srow, pvp[64:65, :, :])
                    rsp = npool.tile([8, 128], F32, tag="rsp")
                    nc.sync.dma_start(out=rsp, in_=srow)
                    rcp = npool.tile([8, 128], F32, tag="rcp")
                    nc.vector.reciprocal_approx_fast(out=rcp, in_=rsp)
                    rc16 = npool.tile([8, 128], F16, tag="rc16")
                    nc.vector.tensor_copy(rc16, rcp)
                    r0 = npool.tile([1, 1024], F16, tag="r0")
                    nc.sync.dma_start(out=r0, in_=rc16)
                    bc = npool.tile([64, 1024], F16, tag="bc")
                    nc.gpsimd.partition_broadcast(bc, r0)
                    nc.vector.tensor_tensor(out=aT[hp][0:64, sl],
                                            in0=pvp[0:64, 0, :], in1=bc[:, 0:512],
                                            op=OP.mult)
                    stag = npool.tile([64, 512], F16, tag="stag")
                    nc.vector.tensor_tensor(out=stag, in0=pvp[0:64, 1, :],
                                            in1=bc[:, 512:1024], op=OP.mult)
                    nc.sync.dma_start(out=aT[hp][64:128, sl], in_=stag)

            def oproj(s):
                for j in range(4):
                    st = 4 * s + j
                    psos = [pmm.tile([128, 512], F32, tag="mm",
                                     name=f"o{st}_{d_}") for d_ in range(2)]
                    for ch in range(2):
                        for dsp in range(2):
                            nc.tensor.matmul(psos[dsp], aT[ch][:, bass.ts(st, 128)],
                                             wo_sb[:, ch, bass.ts(dsp, 512)],
                                             start=(ch == 0), stop=(ch == 1))
                    ost = opool.tile([128, 1024], F16, tag="o")
                    nc.scalar.copy(ost[:, 0:512], psos[0])
                    nc.vector.tensor_copy(ost[:, 512:1024], psos[1])
                    nc.sync.dma_start(out=o_part[bass.ts(st, 128), :], in_=ost)

            for i in range(6):
                if i <= 3:
                    proj_strip(i)
                if 1 <= i <= 4:
                    scores_kts(i - 1)
                    pv_norm(i - 1)
                if i >= 2:
                    oproj(i - 2)

    nc.compile()
    return nc


_NC = {}


def _get_nc():
    if "k" not in _NC:
        _NC["k"] = _build()
    return _NC["k"]


def _host_inputs(hidden_states, position_ids, Wq, Wk, Wv, Wo):
    hs = np.asarray(hidden_states, np.float32)
    Wq = np.asarray(Wq, np.float32)
    Wk = np.asarray(Wk, np.float32)
    Wv = np.asarray(Wv, np.float32)
    Wo = np.asarray(Wo, np.float32)

    hsT4 = []
    for b in range(B):
        hT = np.ascontiguousarray(hs[b].T).astype(np.float16)      # [D, S]
        # [NST, 8, 128, 512]: strip-major, d-chunk, d-row, position
        h4 = np.empty((NST, 8, 128, 512), np.float16)
        for s_ in range(NST):
            for dt in range(8):
                h4[s_, dt] = hT[128 * dt:128 * (dt + 1), 512 * s_:512 * (s_ + 1)]
        hsT4.append(np.ascontiguousarray(h4))

    inv_freq = (1.0 / (THETA ** (np.arange(0, HD, 2, dtype=np.float32) / HD))).astype(np.float32)
    cos2, sin2 = [], []
    for b in range(B):
        pos = np.asarray(position_ids[b]).astype(np.float32)
        freqs = pos[:, None] * inv_freq[None, :]          # [S, 32]
        cosf = np.cos(freqs).T                            # [32, S]
        sinf = np.sin(freqs).T
        cos64 = np.concatenate([cosf, cosf], axis=0)      # [64, S]
        sin64s = np.concatenate([-sinf, sinf], axis=0)    # sign-folded
        cos2.append(np.concatenate([cos64, cos64], axis=0).astype(np.float16))
        sin2.append(np.concatenate([sin64s, sin64s], axis=0).astype(np.float16))

    p = np.arange(128)[:, None]
    c = np.arange(WIN)[None, :]
    bandmask = ((p <= c) & (c < p + WINDOW)).astype(np.float16)

    perm = np.zeros((64, 64), np.float32)
    for i in range(32):
        perm[i, i + 32] = 1.0
        perm[i + 32, i] = 1.0
    perm2 = np.kron(np.eye(2, dtype=np.float32), perm)    # [128, 128]
    permT = np.ascontiguousarray(perm2.T).astype(np.float16)

    in_maps = []
    for core in range(8):
        b, g = divmod(core, 4)
        wqT = np.ascontiguousarray(Wq[256 * g:256 * (g + 1)].T).astype(np.float16).reshape(8, 128, 256)
        wkT = np.ascontiguousarray(Wk[64 * g:64 * (g + 1)].T).astype(np.float16).reshape(8, 128, 64)
        wvT = np.ascontiguousarray(Wv[64 * g:64 * (g + 1)].T).astype(np.float16).reshape(8, 128, 64)
        woT = np.ascontiguousarray(Wo[:, 256 * g:256 * (g + 1)].T).astype(np.float16).reshape(2, 128, D)
        in_maps.append({
            "hsT4": hsT4[b], "wqT": wqT, "wkT": wkT, "wvT": wvT, "woT": woT,
            "cos2": cos2[b], "sinS2": sin2[b],
            "bandmask": bandmask, "permT": permT,
        })
    return in_maps


def run_spmd(hidden_states, attention_mask, position_ids, Wq, Wk, Wv, Wo, **spmd_kwargs):
    nc = _get_nc()
    in_maps = _host_inputs(hidden_states, position_ids, Wq, Wk, Wv, Wo)
    res = run_bass_kernel_spmd(nc, in_maps, list(range(8)), **spmd_kwargs)
    out = np.zeros((B, S, D), np.float32)
    for core in range(8):
        out[core // 4] += np.asarray(res.results[core]["o_part"], np.float32)
    return out, res


def kernel(hidden_states, attention_mask, position_ids, Wq, Wk, Wv, Wo):
    out, _ = run_spmd(hidden_states, attention_mask, position_ids, Wq, Wk, Wv, Wo)
    return out


# revision 6
# speedup vs baseline: 1.0437x; 1.0437x over previous
"""Mimi-style GQA attention (RoPE + 250-wide sliding causal window) on 8 TRN2 NeuronCores.

Sharding: core c handles batch b=c//4 and KV-head group g=c%4 (4 query heads +
1 KV head). Wq/Wk/Wv column-sharded, Wo row-sharded along the head dim; each
core emits a partial [S, D] f16 output; host sums the 4 partials per batch.

Strip-streamed pipeline (4 q-strips of 512, software-pipelined over 6
iterations) keeps the PE matmul queue dense so the HAM clock gate stays open:
  iter i: proj+rope strip i | scores for strip i-1's k-tiles | PV+norm for
  q-strip i-1 | output projection for strip i-2.
Implementation notes:
  - hs streamed per strip; projections start as soon as strip 0 lands.
  - One rotating PSUM pool of 4x [128,2,512] 2-bank slots (8 banks total)
    serves q-proj, k+v, rope-rot, score pairs, PV and O-proj tiles; pairing
    two heads per slot halves the PE->ACT handoff count.
  - V is projected directly in [pos, hd] orientation (hs chunk stationary),
    so no PE transposes; a ones column in vaug yields softmax denominators
    for free in PV row 64.
  - PV accumulation needs no zero-seed matmuls: pieces ascend in kt, so the
    first piece (start=True) clears the bank and later start=False matmuls
    accumulate where has_written is set and overwrite fresh columns.
  - PV psum is evacuated to SBUF (f16) immediately, freeing the slot in
    <1us; the softmax-normalization chain (DMA reshape -> DVE
    reciprocal_approx_fast -> DMA -> gpsimd partition broadcast -> DVE
    multiplies) runs off the SBUF copy one iteration ahead of the O-proj.
"""
import os
import sys

for _p in ("/opt/trn_rl_repo", "/root/.axon_site/_ro/trn_rl_repo"):
    if os.path.isdir(_p) and _p not in sys.path:
        sys.path.append(_p)

import numpy as np
import concourse.bass as bass
import concourse.mybir as mybir
import concourse.tile as tile
from concourse import bacc
from concourse.bass_utils import run_bass_kernel_spmd

F32 = mybir.dt.float32
F16 = mybir.dt.float16
AF = mybir.ActivationFunctionType
OP = mybir.AluOpType

B, S, D = 2, 2048, 1024
H, HK, HD = 16, 4, 64
WINDOW = 250
SCALE = 1.0 / np.sqrt(HD)
THETA = 10000.0
NKT = S // 128          # 16 k-tiles
NST = S // 512          # 4 q-strips
WIN = 384               # padded per-k-tile q-window


def _pv_pieces(s):
    """PV pieces for q-strip s, ascending kt: list of (kt, c0, c1) ranges."""
    out = []
    for kt in range(max(0, 4 * s - 2), min(NKT - 1, 4 * s + 3) + 1):
        j0 = 128 * kt
        w = min(WIN, S - j0)
        c_lo = max(0, 512 * s - j0)
        c_hi = min(w, 512 * (s + 1) - j0)
        if c_lo < c_hi:
            out.append((kt, c_lo, c_hi))
    return out


def _build():
    nc = bacc.Bacc(None, target_bir_lowering=False)

    hsT4 = nc.declare_dram_parameter("hsT4", [NST, 8, 128, 512], F16, isOutput=False)
    wq = nc.declare_dram_parameter("wqT", [8, 128, 256], F16, isOutput=False)
    wk = nc.declare_dram_parameter("wkT", [8, 128, 64], F16, isOutput=False)
    wv = nc.declare_dram_parameter("wvT", [8, 128, 64], F16, isOutput=False)
    wo = nc.declare_dram_parameter("woT", [2, 128, D], F16, isOutput=False)
    cosd = nc.declare_dram_parameter("cos2", [128, S], F16, isOutput=False)
    sind = nc.declare_dram_parameter("sinS2", [128, S], F16, isOutput=False)
    maskd = nc.declare_dram_parameter("bandmask", [128, 2, 512], F16, isOutput=False)
    permd = nc.declare_dram_parameter("permT", [128, 128], F16, isOutput=False)
    o_part = nc.declare_dram_parameter("o_part", [S, D], F16, isOutput=True)

    with tile.TileContext(nc) as tc:
        with (
            tc.tile_pool(name="persist", bufs=1) as pp,
            tc.tile_pool(name="hts", bufs=4) as hpool,
            tc.tile_pool(name="work", bufs=2) as wkp,
            tc.tile_pool(name="expm", bufs=18) as ep,
            tc.tile_pool(name="norm", bufs=3) as npool,
            tc.tile_pool(name="ost", bufs=3) as opool,
            tc.tile_pool(name="pmm", bufs=4, space="PSUM") as pmm,
        ):
            # ---- persistent SBUF tiles ----
            warm = pp.tile([128, 512], F16, tag="warm")
            nc.vector.memset(warm, 0.0)

            wq_sb = pp.tile([128, 8, 256], F16, tag="wq")
            wk_sb = pp.tile([128, 8, 64], F16, tag="wk")
            wv_sb = pp.tile([128, 8, 64], F16, tag="wv")
            wo_sb = pp.tile([128, 2, D], F16, tag="wo")
            cos_sb = pp.tile([128, S], F16, tag="cos")
            sin_sb = pp.tile([128, S], F16, tag="sin")
            mask_sb = pp.tile([128, 2, 512], F16, tag="mask")
            perm_sb = pp.tile([128, 128], F16, tag="perm")

            qT = [pp.tile([128, S], F16, tag=f"qT{m}", name=f"qT{m}") for m in range(2)]
            kdup = pp.tile([128, S], F16, tag="kdup")
            vaug = pp.tile([128, NKT, 65], F16, tag="vaug")
            nc.vector.memset(vaug[:, :, 64:65], 1.0)
            aT = [pp.tile([128, S], F16, tag=f"aT{m}", name=f"aT{m}") for m in range(2)]

            # ---- input DMA stream (ordered on the sync queue) ----
            nc.sync.dma_start(out=wq_sb, in_=wq.rearrange("a p c -> p a c"))
            nc.sync.dma_start(out=wk_sb, in_=wk.rearrange("a p c -> p a c"))
            nc.sync.dma_start(out=wv_sb, in_=wv.rearrange("a p c -> p a c"))
            nc.sync.dma_start(out=perm_sb, in_=permd[:, :])
            nc.sync.dma_start(out=mask_sb, in_=maskd[:, :, :])
            ht = []
            for s_ in range(NST):
                t = hpool.tile([128, 8, 512], F16, tag="ht", name=f"ht{s_}")
                nc.sync.dma_start(out=t, in_=hsT4[s_].rearrange("a p c -> p a c"))
                ht.append(t)
                if s_ == 0:
                    nc.sync.dma_start(out=cos_sb, in_=cosd[:, :])
                    nc.sync.dma_start(out=sin_sb, in_=sind[:, :])
                if s_ == 1:
                    nc.sync.dma_start(out=wo_sb, in_=wo.rearrange("a p c -> p a c"))

            # ---- PE warm-up (no DMA deps; ramps HAM while inputs stream) ----
            for _w in range(16):
                wmm = pmm.tile([128, 2, 512], F32, tag="mm", name=f"warm{_w}")
                nc.tensor.matmul(wmm[:, 0, :], warm[:, 0:128], warm,
                                 start=True, stop=True)

            expm = {}

            def proj_strip(s):
                sl = bass.ts(s, 512)
                # q projections (m = head-pair of the weight columns)
                qps = pmm.tile([128, 2, 512], F32, tag="mm", name=f"q{s}")
                raws = []
                for m in range(2):
                    for dt in range(8):
                        nc.tensor.matmul(qps[:, m, :], wq_sb[:, dt, bass.ts(m, 128)],
                                         ht[s][:, dt, :],
                                         start=(dt == 0), stop=(dt == 7))
                    raw = wkp.tile([128, 512], F16, tag=f"raw{m}")
                    nc.scalar.copy(raw, qps[:, m, :])
                    raws.append(raw)
                # k projection (64 hd rows) + v projection (direct [pos, hd])
                kvps = pmm.tile([128, 2, 512], F32, tag="mm", name=f"kv{s}")
                for dt in range(8):
                    nc.tensor.matmul(kvps[0:64, 0, :], wk_sb[:, dt, :],
                                     ht[s][:, dt, :],
                                     start=(dt == 0), stop=(dt == 7))
                rawk = wkp.tile([64, 512], F16, tag="rawk")
                nc.scalar.copy(rawk, kvps[0:64, 0, :])
                for pt in range(4):
                    for dt in range(8):
                        nc.tensor.matmul(kvps[:, 1, bass.ts(pt, 64)],
                                         ht[s][:, dt, bass.ts(pt, 128)],
                                         wv_sb[:, dt, :],
                                         start=(pt == 0 and dt == 0),
                                         stop=(pt == 3 and dt == 7),
                                         skip_group_check=True)
                nc.vector.tensor_copy(vaug[:, 4 * s:4 * s + 4, 0:64], kvps[:, 1, 0:256])
                # rope rotations + combines
                rot = pmm.tile([128, 2, 512], F32, tag="mm", name=f"rot{s}")
                for m in range(2):
                    nc.tensor.matmul(rot[:, m, :], perm_sb, raws[m],
                                     start=True, stop=True)
                rotk = pmm.tile([128, 2, 512], F32, tag="mm", name=f"rotk{s}")
                nc.tensor.matmul(rotk[0:64, 0, :], perm_sb[0:64, 0:64], rawk,
                                 start=True, stop=True)
                for m in range(2):
                    t1 = wkp.tile([128, 512], F16, tag=f"t1{m}")
                    nc.vector.tensor_tensor(out=t1, in0=rot[:, m, :],
                                            in1=sin_sb[:, sl], op=OP.mult)
                    t2 = wkp.tile([128, 512], F16, tag=f"t2{m}")
                    nc.gpsimd.tensor_tensor(out=t2, in0=raws[m], in1=cos_sb[:, sl],
                                            op=OP.mult)
                    nc.vector.tensor_tensor(out=qT[m][:, sl], in0=t1, in1=t2,
                                            op=OP.add)
                t1k = wkp.tile([64, 512], F16, tag="t1k")
                nc.vector.tensor_tensor(out=t1k, in0=rotk[0:64, 0, :],
                                        in1=sin_sb[0:64, sl], op=OP.mult)
                t2k = wkp.tile([64, 512], F16, tag="t2k")
                nc.gpsimd.tensor_tensor(out=t2k, in0=rawk, in1=cos_sb[0:64, sl],
                                        op=OP.mult)
                nc.vector.tensor_tensor(out=kdup[0:64, sl], in0=t1k, in1=t2k,
                                        op=OP.add)
                nc.sync.dma_start(out=kdup[64:128, sl], in_=kdup[0:64, sl])

            def scores_kts(s):
                """Scores + exp + band-mask for the 4 k-tiles of strip s.

                Each psum slot holds one (kt, hp) head-pair: hh0 in bank 0,
                hh1 in bank 1, so exp and mask run once per pair."""
                for kt in range(4 * s, 4 * s + 4):
                    j0 = 128 * kt
                    w = min(WIN, S - j0)
                    for hp in range(2):
                        pss = pmm.tile([128, 2, 512], F32, tag="mm",
                                       name=f"sc{kt}_{hp}")
                        for hh in range(2):
                            h = 2 * hp + hh
                            m, half = h // 2, (h % 2) * 64
                            nc.tensor.matmul(
                                pss[:, hh, 0:w],
                                kdup[half:half + 64, bass.ts(kt, 128)],
                                qT[m][half:half + 64, j0:j0 + w],
                                start=True, stop=True)
                        et = ep.tile([128, 2, 512], F16, tag="e")
                        for hh in range(2):
                            nc.scalar.activation(et[:, hh, 0:w], pss[:, hh, 0:w],
                                                 AF.Exp, scale=float(SCALE))
                        if kt % 4 != 3:
                            nc.vector.tensor_tensor(out=et[:, :, 0:w],
                                                    in0=et[:, :, 0:w],
                                                    in1=mask_sb[:, :, 0:w],
                                                    op=OP.mult)
                        else:
                            nc.gpsimd.tensor_tensor(out=et[:, :, 0:w],
                                                    in0=et[:, :, 0:w],
                                                    in1=mask_sb[:, :, 0:w],
                                                    op=OP.mult)
                        expm[(kt, hp)] = et

            def pv_norm(s):
                pieces = _pv_pieces(s)
                sl = bass.ts(s, 512)
                for hp in range(2):
                    pvp = pmm.tile([128, 2, 512], F32, tag="mm", name=f"pv{s}_{hp}")
                    for i, (kt, c0, c1) in enumerate(pieces):
                        base = 128 * kt + c0 - 512 * s
                        for hh in range(2):
                            nc.tensor.matmul(pvp[0:65, hh, base:base + (c1 - c0)],
                                             vaug[:, kt, 0:65],
                                             expm[(kt, hp)][:, hh, c0:c1],
                                             start=(i == 0), stop=(i == len(pieces) - 1),
                                             skip_group_check=True)
                    # evacuate psum immediately; the slot frees in <1us
                    pvs = npool.tile([65, 2, 512], F16, tag=f"pvs{hp}")
                    nc.vector.tensor_copy(pvs, pvp[0:65, :, :])
                    # softmax denominators live in row 64 (ones column of vaug)
                    rsp16 = npool.tile([8, 128], F16, tag="rsp16")
                    nc.sync.dma_start(out=rsp16, in_=pvs[64:65, :, :])
                    rspf = npool.tile([8, 128], F32, tag="rspf")
                    nc.vector.tensor_copy(rspf, rsp16)
                    rcp = npool.tile([8, 128], F32, tag="rcp")
                    nc.vector.reciprocal_approx_fast(out=rcp, in_=rspf)
                    rc16 = npool.tile([8, 128], F16, tag="rc16")
                    nc.vector.tensor_copy(rc16, rcp)
                    r0 = npool.tile([1, 1024], F16, tag="r0")
                    nc.sync.dma_start(out=r0, in_=rc16)
                    bc = npool.tile([64, 1024], F16, tag="bc")
                    nc.gpsimd.partition_broadcast(bc, r0)
                    nc.vector.tensor_tensor(out=aT[hp][0:64, sl],
                                            in0=pvs[0:64, 0, :], in1=bc[:, 0:512],
                                            op=OP.mult)
                    stag = npool.tile([64, 512], F16, tag="stag")
                    nc.vector.tensor_tensor(out=stag, in0=pvs[0:64, 1, :],
                                            in1=bc[:, 512:1024], op=OP.mult)
                    nc.sync.dma_start(out=aT[hp][64:128, sl], in_=stag)

            def oproj(s):
                for j in range(4):
                    st = 4 * s + j
                    psos = pmm.tile([128, 2, 512], F32, tag="mm", name=f"o{st}")
                    for ch in range(2):
                        for dsp in range(2):
                            nc.tensor.matmul(psos[:, dsp, :],
                                             aT[ch][:, bass.ts(st, 128)],
                                             wo_sb[:, ch, bass.ts(dsp, 512)],
                                             start=(ch == 0), stop=(ch == 1))
                    ost = opool.tile([128, 1024], F16, tag="o")
                    nc.scalar.copy(ost[:, 0:512], psos[:, 0, :])
                    nc.vector.tensor_copy(ost[:, 512:1024], psos[:, 1, :])
                    nc.sync.dma_start(out=o_part[bass.ts(st, 128), :], in_=ost)

            for i in range(6):
                if i <= 3:
                    proj_strip(i)
                if 1 <= i <= 4:
                    scores_kts(i - 1)
                    pv_norm(i - 1)
                if i >= 2:
                    oproj(i - 2)

    nc.compile()
    return nc


_NC = {}


def _get_nc():
    if "k" not in _NC:
        _NC["k"] = _build()
    return _NC["k"]


def _host_inputs(hidden_states, position_ids, Wq, Wk, Wv, Wo):
    hs = np.asarray(hidden_states, np.float32)
    Wq = np.asarray(Wq, np.float32)
    Wk = np.asarray(Wk, np.float32)
    Wv = np.asarray(Wv, np.float32)
    Wo = np.asarray(Wo, np.float32)

    hsT4 = []
    for b in range(B):
        hT = np.ascontiguousarray(hs[b].T).astype(np.float16)      # [D, S]
        h4 = np.empty((NST, 8, 128, 512), np.float16)
        for s_ in range(NST):
            for dt in range(8):
                h4[s_, dt] = hT[128 * dt:128 * (dt + 1), 512 * s_:512 * (s_ + 1)]
        hsT4.append(np.ascontiguousarray(h4))

    inv_freq = (1.0 / (THETA ** (np.arange(0, HD, 2, dtype=np.float32) / HD))).astype(np.float32)
    cos2, sin2 = [], []
    for b in range(B):
        pos = np.asarray(position_ids[b]).astype(np.float32)
        freqs = pos[:, None] * inv_freq[None, :]          # [S, 32]
        cosf = np.cos(freqs).T                            # [32, S]
        sinf = np.sin(freqs).T
        cos64 = np.concatenate([cosf, cosf], axis=0)      # [64, S]
        sin64s = np.concatenate([-sinf, sinf], axis=0)    # sign-folded
        cos2.append(np.concatenate([cos64, cos64], axis=0).astype(np.float16))
        sin2.append(np.concatenate([sin64s, sin64s], axis=0).astype(np.float16))

    p = np.arange(128)[:, None]
    c = np.arange(WIN)[None, :]
    band = ((p <= c) & (c < p + WINDOW)).astype(np.float16)   # [128, 384]
    mask2 = np.zeros((128, 2, 512), np.float16)
    mask2[:, 0, 0:WIN] = band
    mask2[:, 1, 0:WIN] = band

    perm = np.zeros((64, 64), np.float32)
    for i in range(32):
        perm[i, i + 32] = 1.0
        perm[i + 32, i] = 1.0
    perm2 = np.kron(np.eye(2, dtype=np.float32), perm)    # [128, 128]
    permT = np.ascontiguousarray(perm2.T).astype(np.float16)

    in_maps = []
    for core in range(8):
        b, g = divmod(core, 4)
        wqT = np.ascontiguousarray(Wq[256 * g:256 * (g + 1)].T).astype(np.float16).reshape(8, 128, 256)
        wkT = np.ascontiguousarray(Wk[64 * g:64 * (g + 1)].T).astype(np.float16).reshape(8, 128, 64)
        wvT = np.ascontiguousarray(Wv[64 * g:64 * (g + 1)].T).astype(np.float16).reshape(8, 128, 64)
        woT = np.ascontiguousarray(Wo[:, 256 * g:256 * (g + 1)].T).astype(np.float16).reshape(2, 128, D)
        in_maps.append({
            "hsT4": hsT4[b], "wqT": wqT, "wkT": wkT, "wvT": wvT, "woT": woT,
            "cos2": cos2[b], "sinS2": sin2[b],
            "bandmask": mask2, "permT": permT,
        })
    return in_maps


def run_spmd(hidden_states, attention_mask, position_ids, Wq, Wk, Wv, Wo, **spmd_kwargs):
    nc = _get_nc()
    in_maps = _host_inputs(hidden_states, position_ids, Wq, Wk, Wv, Wo)
    res = run_bass_kernel_spmd(nc, in_maps, list(range(8)), **spmd_kwargs)
    out = np.zeros((B, S, D), np.float32)
    for core in range(8):
        out[core // 4] += np.asarray(res.results[core]["o_part"], np.float32)
    return out, res


def kernel(hidden_states, attention_mask, position_ids, Wq, Wk, Wv, Wo):
    out, _ = run_spmd(hidden_states, attention_mask, position_ids, Wq, Wk, Wv, Wo)
    return out


# revision 11
# speedup vs baseline: 1.1525x; 1.1042x over previous
"""Mimi-style GQA attention (RoPE + 250-wide sliding causal window) on 8 TRN2 NeuronCores.

Sharding: core c handles batch b=c//4 and KV-head group g=c%4 (4 query heads +
1 KV head). Wq/Wk/Wv column-sharded, Wo row-sharded along the head dim; each
core emits a partial [S, D] f16 output; host sums the 4 partials per batch.

Strip-streamed pipeline (4 q-strips of 512, software-pipelined over 6
iterations) keeps the PE matmul queue dense so the HAM clock gate stays open:
  iter i: proj+rope strip i | scores for strip i-1's k-tiles | PV+norm for
  q-strip i-1 | output projection for strip i-2.
Implementation notes:
  - hs streamed per strip; projections start as soon as strip 0 lands.
  - One rotating PSUM pool of 4x [128,2,512] 2-bank slots (8 banks total)
    serves q-proj, k+v, rope-rot, score pairs, PV and O-proj tiles; pairing
    two heads per slot halves the PE->ACT handoff count.
  - V is projected directly in [pos, hd] orientation (hs chunk stationary),
    so no PE transposes; a ones column in vaug yields softmax denominators
    for free in PV row 64.
  - PV accumulation needs no zero-seed matmuls: pieces ascend in kt, so the
    first piece (start=True) clears the bank and later start=False matmuls
    accumulate where has_written is set and overwrite fresh columns.
  - PV psum is evacuated to SBUF (f16) immediately, freeing the slot in
    <1us; the softmax-normalization chain (DMA reshape -> DVE
    reciprocal_approx_fast -> DMA -> gpsimd partition broadcast -> DVE
    multiplies) runs off the SBUF copy one iteration ahead of the O-proj.
"""
import os
import sys

for _p in ("/opt/trn_rl_repo", "/root/.axon_site/_ro/trn_rl_repo"):
    if os.path.isdir(_p) and _p not in sys.path:
        sys.path.append(_p)

import numpy as np
import concourse.bass as bass
import concourse.mybir as mybir
import concourse.tile as tile
from concourse import bacc
from concourse.bass_utils import run_bass_kernel_spmd

F32 = mybir.dt.float32
F16 = mybir.dt.float16
AF = mybir.ActivationFunctionType
OP = mybir.AluOpType

B, S, D = 2, 2048, 1024
H, HK, HD = 16, 4, 64
WINDOW = 250
SCALE = 1.0 / np.sqrt(HD)
THETA = 10000.0
NKT = S // 128          # 16 k-tiles
NST = S // 512          # 4 q-strips
WIN = 384               # padded per-k-tile q-window


def _pv_pieces(s):
    """PV pieces for q-strip s, ascending kt: list of (kt, c0, c1) ranges."""
    out = []
    for kt in range(max(0, 4 * s - 2), min(NKT - 1, 4 * s + 3) + 1):
        j0 = 128 * kt
        w = min(WIN, S - j0)
        c_lo = max(0, 512 * s - j0)
        c_hi = min(w, 512 * (s + 1) - j0)
        if c_lo < c_hi:
            out.append((kt, c_lo, c_hi))
    return out


def _build():
    nc = bacc.Bacc(None, target_bir_lowering=False)

    hsT4 = nc.declare_dram_parameter("hsT4", [NST, 8, 128, 512], F16, isOutput=False)
    wq = nc.declare_dram_parameter("wqT", [8, 128, 256], F16, isOutput=False)
    wk = nc.declare_dram_parameter("wkT", [8, 128, 64], F16, isOutput=False)
    wv = nc.declare_dram_parameter("wvT", [8, 128, 64], F16, isOutput=False)
    wo = nc.declare_dram_parameter("woT", [2, 128, D], F16, isOutput=False)
    cosd = nc.declare_dram_parameter("cos2", [128, S], F16, isOutput=False)
    sind = nc.declare_dram_parameter("sinS2", [128, S], F16, isOutput=False)
    maskd = nc.declare_dram_parameter("bandmask", [128, 2, 512], F16, isOutput=False)
    permd = nc.declare_dram_parameter("permT", [128, 128], F16, isOutput=False)
    o_part = nc.declare_dram_parameter("o_part", [S, D], F16, isOutput=True)

    with tile.TileContext(nc) as tc:
        with (
            tc.tile_pool(name="persist", bufs=1) as pp,
            tc.tile_pool(name="hts", bufs=4) as hpool,
            tc.tile_pool(name="work", bufs=2) as wkp,
            tc.tile_pool(name="expm", bufs=18) as ep,
            tc.tile_pool(name="norm", bufs=3) as npool,
            tc.tile_pool(name="ost", bufs=3) as opool,
            tc.tile_pool(name="pmm", bufs=4, space="PSUM") as pmm,
        ):
            # ---- persistent SBUF tiles ----
            warm = pp.tile([128, 512], F16, tag="warm")
            nc.vector.memset(warm, 0.0)

            wq_sb = pp.tile([128, 8, 256], F16, tag="wq")
            wk_sb = pp.tile([128, 8, 64], F16, tag="wk")
            wv_sb = pp.tile([128, 8, 64], F16, tag="wv")
            wo_sb = pp.tile([128, 2, D], F16, tag="wo")
            cos_sb = pp.tile([128, S], F16, tag="cos")
            sin_sb = pp.tile([128, S], F16, tag="sin")
            mask_sb = pp.tile([128, 2, 512], F16, tag="mask")
            perm_sb = pp.tile([128, 128], F16, tag="perm")

            qT = [pp.tile([128, S], F16, tag=f"qT{m}", name=f"qT{m}") for m in range(2)]
            kdup = pp.tile([128, S], F16, tag="kdup")
            vaug = pp.tile([128, NKT, 65], F16, tag="vaug")
            nc.vector.memset(vaug[:, :, 64:65], 1.0)
            aT = [pp.tile([128, S], F16, tag=f"aT{m}", name=f"aT{m}") for m in range(2)]

            # ---- input DMA stream (ordered on the sync queue) ----
            nc.sync.dma_start(out=wq_sb, in_=wq.rearrange("a p c -> p a c"))
            nc.sync.dma_start(out=wk_sb, in_=wk.rearrange("a p c -> p a c"))
            nc.sync.dma_start(out=wv_sb, in_=wv.rearrange("a p c -> p a c"))
            nc.sync.dma_start(out=perm_sb, in_=permd[:, :])
            nc.sync.dma_start(out=mask_sb, in_=maskd[:, :, :])
            ht = []
            for s_ in range(NST):
                t = hpool.tile([128, 8, 512], F16, tag="ht", name=f"ht{s_}")
                nc.sync.dma_start(out=t, in_=hsT4[s_].rearrange("a p c -> p a c"))
                ht.append(t)
                if s_ == 0:
                    nc.sync.dma_start(out=cos_sb, in_=cosd[:, :])
                    nc.sync.dma_start(out=sin_sb, in_=sind[:, :])
                if s_ == 1:
                    nc.sync.dma_start(out=wo_sb, in_=wo.rearrange("a p c -> p a c"))

            # ---- PE warm-up (no DMA deps; ramps HAM while inputs stream) ----
            for _w in range(16):
                wmm = pmm.tile([128, 2, 512], F32, tag="mm", name=f"warm{_w}")
                nc.tensor.matmul(wmm[:, 0, :], warm[:, 0:128], warm,
                                 start=True, stop=True)

            expm = {}

            def proj_strip(s):
                sl = bass.ts(s, 512)
                # q projections (m = head-pair of the weight columns)
                qps = pmm.tile([128, 2, 512], F32, tag="mm", name=f"q{s}")
                raws = []
                for m in range(2):
                    for dt in range(8):
                        nc.tensor.matmul(qps[:, m, :], wq_sb[:, dt, bass.ts(m, 128)],
                                         ht[s][:, dt, :],
                                         start=(dt == 0), stop=(dt == 7))
                    raw = wkp.tile([128, 512], F16, tag=f"raw{m}")
                    nc.scalar.copy(raw, qps[:, m, :])
                    raws.append(raw)
                # k projection (64 hd rows) + v projection (direct [pos, hd])
                kvps = pmm.tile([128, 2, 512], F32, tag="mm", name=f"kv{s}")
                for dt in range(8):
                    nc.tensor.matmul(kvps[0:64, 0, :], wk_sb[:, dt, :],
                                     ht[s][:, dt, :],
                                     start=(dt == 0), stop=(dt == 7))
                rawk = wkp.tile([64, 512], F16, tag="rawk")
                nc.scalar.copy(rawk, kvps[0:64, 0, :])
                for pt in range(4):
                    for dt in range(8):
                        nc.tensor.matmul(kvps[:, 1, bass.ts(pt, 64)],
                                         ht[s][:, dt, bass.ts(pt, 128)],
                                         wv_sb[:, dt, :],
                                         start=(pt == 0 and dt == 0),
                                         stop=(pt == 3 and dt == 7),
                                         skip_group_check=True)
                nc.vector.tensor_copy(vaug[:, 4 * s:4 * s + 4, 0:64], kvps[:, 1, 0:256])
                # rope rotations + combines
                rot = pmm.tile([128, 2, 512], F32, tag="mm", name=f"rot{s}")
                for m in range(2):
                    nc.tensor.matmul(rot[:, m, :], perm_sb, raws[m],
                                     start=True, stop=True)
                rotk = pmm.tile([128, 2, 512], F32, tag="mm", name=f"rotk{s}")
                nc.tensor.matmul(rotk[0:64, 0, :], perm_sb[0:64, 0:64], rawk,
                                 start=True, stop=True)
                for m in range(2):
                    t1 = wkp.tile([128, 512], F16, tag=f"t1{m}")
                    nc.vector.tensor_tensor(out=t1, in0=rot[:, m, :],
                                            in1=sin_sb[:, sl], op=OP.mult)
                    t2 = wkp.tile([128, 512], F16, tag=f"t2{m}")
                    nc.gpsimd.tensor_tensor(out=t2, in0=raws[m], in1=cos_sb[:, sl],
                                            op=OP.mult)
                    nc.vector.tensor_tensor(out=qT[m][:, sl], in0=t1, in1=t2,
                                            op=OP.add)
                t1k = wkp.tile([64, 512], F16, tag="t1k")
                nc.vector.tensor_tensor(out=t1k, in0=rotk[0:64, 0, :],
                                        in1=sin_sb[0:64, sl], op=OP.mult)
                t2k = wkp.tile([64, 512], F16, tag="t2k")
                nc.gpsimd.tensor_tensor(out=t2k, in0=rawk, in1=cos_sb[0:64, sl],
                                        op=OP.mult)
                nc.vector.tensor_tensor(out=kdup[0:64, sl], in0=t1k, in1=t2k,
                                        op=OP.add)
                nc.gpsimd.dma_start(out=kdup[64:128, sl], in_=kdup[0:64, sl])

            def scores_part(s, part):
                """Scores + exp + band-mask for strip s's k-tiles.

                part 'A': window columns inside strip s (needs only rope(s));
                part 'B': spill-over columns in strip s+1 (needs rope(s+1)).
                The A/B column split at the strip boundary coincides exactly
                with the PV piece boundaries, so q-strip s's PV consumes only
                A-parts of strip s and B-parts of strip s-1 — all available
                one iteration before PV runs."""
                for kt in range(4 * s, 4 * s + 4):
                    j0 = 128 * kt
                    w = min(WIN, S - j0)
                    j = kt % 4
                    c0, c1 = (0, min(w, 512 - 128 * j)) if part == "A" else \
                             (min(w, 512 - 128 * j), w)
                    if c0 >= c1:
                        continue
                    for hp in range(2):
                        if part == "A":
                            pss = pmm.tile([128, 2, 512], F32, tag="mm",
                                           name=f"scA{kt}_{hp}")
                            et = ep.tile([128, 2, 512], F16, tag="e")
                            expm[(kt, hp)] = et
                        else:
                            pss = pmm.tile([128, 2, 512], F32, tag="mm",
                                           name=f"scB{kt}_{hp}")
                            et = expm[(kt, hp)]
                        for hh in range(2):
                            h = 2 * hp + hh
                            m, half = h // 2, (h % 2) * 64
                            nc.tensor.matmul(
                                pss[:, hh, c0:c1],
                                kdup[half:half + 64, bass.ts(kt, 128)],
                                qT[m][half:half + 64, j0 + c0:j0 + c1],
                                start=True, stop=True)
                        for hh in range(2):
                            nc.scalar.activation(et[:, hh, c0:c1], pss[:, hh, c0:c1],
                                                 AF.Exp, scale=float(SCALE))
                        if part == "A":
                            nc.vector.tensor_tensor(out=et[:, :, c0:c1],
                                                    in0=et[:, :, c0:c1],
                                                    in1=mask_sb[:, :, c0:c1],
                                                    op=OP.mult)
                        else:
                            nc.gpsimd.tensor_tensor(out=et[:, :, c0:c1],
                                                    in0=et[:, :, c0:c1],
                                                    in1=mask_sb[:, :, c0:c1],
                                                    op=OP.mult)

            def pv_norm(s):
                pieces = _pv_pieces(s)
                sl = bass.ts(s, 512)
                for hp in range(2):
                    pvp = pmm.tile([128, 2, 512], F32, tag="mm", name=f"pv{s}_{hp}")
                    for i, (kt, c0, c1) in enumerate(pieces):
                        base = 128 * kt + c0 - 512 * s
                        for hh in range(2):
                            nc.tensor.matmul(pvp[0:65, hh, base:base + (c1 - c0)],
                                             vaug[:, kt, 0:65],
                                             expm[(kt, hp)][:, hh, c0:c1],
                                             start=(i == 0), stop=(i == len(pieces) - 1),
                                             skip_group_check=True)
                    # evacuate psum immediately; the slot frees in <1us
                    pvs = npool.tile([65, 2, 512], F16, tag=f"pvs{hp}")
                    nc.vector.tensor_copy(pvs, pvp[0:65, :, :])
                    # softmax denominators live in row 64 (ones column of vaug)
                    rsp16 = npool.tile([8, 128], F16, tag="rsp16")
                    nc.scalar.dma_start(out=rsp16, in_=pvs[64:65, :, :])
                    rspf = npool.tile([8, 128], F32, tag="rspf")
                    nc.vector.tensor_copy(rspf, rsp16)
                    rcp = npool.tile([8, 128], F32, tag="rcp")
                    nc.vector.reciprocal_approx_fast(out=rcp, in_=rspf)
                    rc16 = npool.tile([8, 128], F16, tag="rc16")
                    nc.vector.tensor_copy(rc16, rcp)
                    r0 = npool.tile([1, 1024], F16, tag="r0")
                    nc.scalar.dma_start(out=r0, in_=rc16)
                    bc = npool.tile([64, 1024], F16, tag="bc")
                    nc.gpsimd.partition_broadcast(bc, r0)
                    nc.vector.tensor_tensor(out=aT[hp][0:64, sl],
                                            in0=pvs[0:64, 0, :], in1=bc[:, 0:512],
                                            op=OP.mult)
                    stag = npool.tile([64, 512], F16, tag="stag")
                    nc.vector.tensor_tensor(out=stag, in0=pvs[0:64, 1, :],
                                            in1=bc[:, 512:1024], op=OP.mult)
                    nc.scalar.dma_start(out=aT[hp][64:128, sl], in_=stag)

            def oproj(s):
                for j in range(4):
                    st = 4 * s + j
                    psos = pmm.tile([128, 2, 512], F32, tag="mm", name=f"o{st}")
                    for ch in range(2):
                        for dsp in range(2):
                            nc.tensor.matmul(psos[:, dsp, :],
                                             aT[ch][:, bass.ts(st, 128)],
                                             wo_sb[:, ch, bass.ts(dsp, 512)],
                                             start=(ch == 0), stop=(ch == 1))
                    ost = opool.tile([128, 1024], F16, tag="o")
                    nc.scalar.copy(ost[:, 0:512], psos[:, 0, :])
                    nc.vector.tensor_copy(ost[:, 512:1024], psos[:, 1, :])
                    nc.sync.dma_start(out=o_part[bass.ts(st, 128), :], in_=ost)

            for i in range(5):
                if i <= 3:
                    proj_strip(i)
                if i >= 1:
                    pv_norm(i - 1)      # inputs complete since iteration i-1
                if i <= 3:
                    scores_part(i, "A")
                if 1 <= i <= 3:
                    scores_part(i - 1, "B")
                if i >= 1:
                    oproj(i - 1)

    nc.compile()
    return nc


_NC = {}


def _get_nc():
    if "k" not in _NC:
        _NC["k"] = _build()
    return _NC["k"]


def _host_inputs(hidden_states, position_ids, Wq, Wk, Wv, Wo):
    hs = np.asarray(hidden_states, np.float32)
    Wq = np.asarray(Wq, np.float32)
    Wk = np.asarray(Wk, np.float32)
    Wv = np.asarray(Wv, np.float32)
    Wo = np.asarray(Wo, np.float32)

    hsT4 = []
    for b in range(B):
        hT = np.ascontiguousarray(hs[b].T).astype(np.float16)      # [D, S]
        h4 = np.empty((NST, 8, 128, 512), np.float16)
        for s_ in range(NST):
            for dt in range(8):
                h4[s_, dt] = hT[128 * dt:128 * (dt + 1), 512 * s_:512 * (s_ + 1)]
        hsT4.append(np.ascontiguousarray(h4))

    inv_freq = (1.0 / (THETA ** (np.arange(0, HD, 2, dtype=np.float32) / HD))).astype(np.float32)
    cos2, sin2 = [], []
    for b in range(B):
        pos = np.asarray(position_ids[b]).astype(np.float32)
        freqs = pos[:, None] * inv_freq[None, :]          # [S, 32]
        cosf = np.cos(freqs).T                            # [32, S]
        sinf = np.sin(freqs).T
        cos64 = np.concatenate([cosf, cosf], axis=0)      # [64, S]
        sin64s = np.concatenate([-sinf, sinf], axis=0)    # sign-folded
        cos2.append(np.concatenate([cos64, cos64], axis=0).astype(np.float16))
        sin2.append(np.concatenate([sin64s, sin64s], axis=0).astype(np.float16))

    p = np.arange(128)[:, None]
    c = np.arange(WIN)[None, :]
    band = ((p <= c) & (c < p + WINDOW)).astype(np.float16)   # [128, 384]
    mask2 = np.zeros((128, 2, 512), np.float16)
    mask2[:, 0, 0:WIN] = band
    mask2[:, 1, 0:WIN] = band

    perm = np.zeros((64, 64), np.float32)
    for i in range(32):
        perm[i, i + 32] = 1.0
        perm[i + 32, i] = 1.0
    perm2 = np.kron(np.eye(2, dtype=np.float32), perm)    # [128, 128]
    permT = np.ascontiguousarray(perm2.T).astype(np.float16)

    in_maps = []
    for core in range(8):
        b, g = divmod(core, 4)
        wqT = np.ascontiguousarray(Wq[256 * g:256 * (g + 1)].T).astype(np.float16).reshape(8, 128, 256)
        wkT = np.ascontiguousarray(Wk[64 * g:64 * (g + 1)].T).astype(np.float16).reshape(8, 128, 64)
        wvT = np.ascontiguousarray(Wv[64 * g:64 * (g + 1)].T).astype(np.float16).reshape(8, 128, 64)
        woT = np.ascontiguousarray(Wo[:, 256 * g:256 * (g + 1)].T).astype(np.float16).reshape(2, 128, D)
        in_maps.append({
            "hsT4": hsT4[b], "wqT": wqT, "wkT": wkT, "wvT": wvT, "woT": woT,
            "cos2": cos2[b], "sinS2": sin2[b],
            "bandmask": mask2, "permT": permT,
        })
    return in_maps


def run_spmd(hidden_states, attention_mask, position_ids, Wq, Wk, Wv, Wo, **spmd_kwargs):
    nc = _get_nc()
    in_maps = _host_inputs(hidden_states, position_ids, Wq, Wk, Wv, Wo)
    res = run_bass_kernel_spmd(nc, in_maps, list(range(8)), **spmd_kwargs)
    out = np.zeros((B, S, D), np.float32)
    for core in range(8):
        out[core // 4] += np.asarray(res.results[core]["o_part"], np.float32)
    return out, res


def kernel(hidden_states, attention_mask, position_ids, Wq, Wk, Wv, Wo):
    out, _ = run_spmd(hidden_states, attention_mask, position_ids, Wq, Wk, Wv, Wo)
    return out


# revision 21
# speedup vs baseline: 1.4347x; 1.2449x over previous
"""Mimi-style GQA attention (RoPE + 250-wide sliding causal window) on 8 TRN2 NeuronCores.

Sharding: core c handles batch b=c//4 and KV-head group g=c%4 (4 query heads +
1 KV head). Wq/Wk/Wv column-sharded, Wo row-sharded along the head dim; each
core emits a partial [S, D] f16 output; host sums the 4 partials per batch.

Strip-streamed pipeline (4 q-strips of 512, software-pipelined over 6
iterations) keeps the PE matmul queue dense so the HAM clock gate stays open:
  iter i: proj+rope strip i | scores for strip i-1's k-tiles | PV+norm for
  q-strip i-1 | output projection for strip i-2.
Implementation notes:
  - hs streamed per strip; projections start as soon as strip 0 lands.
  - One rotating PSUM pool of 4x [128,2,512] 2-bank slots (8 banks total)
    serves q-proj, k+v, rope-rot, score pairs, PV and O-proj tiles; pairing
    two heads per slot halves the PE->ACT handoff count.
  - V is projected directly in [pos, hd] orientation (hs chunk stationary),
    so no PE transposes; a ones column in vaug yields softmax denominators
    for free in PV row 64.
  - PV accumulation needs no zero-seed matmuls: pieces ascend in kt, so the
    first piece (start=True) clears the bank and later start=False matmuls
    accumulate where has_written is set and overwrite fresh columns.
  - PV psum is evacuated to SBUF (f16) immediately, freeing the slot in
    <1us; the softmax-normalization chain (DMA reshape -> DVE
    reciprocal_approx_fast -> DMA -> gpsimd partition broadcast -> DVE
    multiplies) runs off the SBUF copy one iteration ahead of the O-proj.
"""
import os
import sys

for _p in ("/opt/trn_rl_repo", "/root/.axon_site/_ro/trn_rl_repo"):
    if os.path.isdir(_p) and _p not in sys.path:
        sys.path.append(_p)

import numpy as np
import concourse.bass as bass
import concourse.mybir as mybir
import concourse.tile as tile
from concourse import bacc
from concourse.bass_utils import run_bass_kernel_spmd

F32 = mybir.dt.float32
F16 = mybir.dt.float16
AF = mybir.ActivationFunctionType
OP = mybir.AluOpType

B, S, D = 2, 2048, 1024
H, HK, HD = 16, 4, 64
WINDOW = 250
SCALE = 1.0 / np.sqrt(HD)
THETA = 10000.0
NKT = S // 128          # 16 k-tiles
NST = S // 512          # 4 q-strips
WIN = 384               # padded per-k-tile q-window


def _pv_pieces(s):
    """PV pieces for q-strip s, ascending kt: list of (kt, c0, c1) ranges."""
    out = []
    for kt in range(max(0, 4 * s - 2), min(NKT - 1, 4 * s + 3) + 1):
        j0 = 128 * kt
        w = min(WIN, S - j0)
        c_lo = max(0, 512 * s - j0)
        c_hi = min(w, 512 * (s + 1) - j0)
        if c_lo < c_hi:
            out.append((kt, c_lo, c_hi))
    return out


def _build():
    nc = bacc.Bacc(None, target_bir_lowering=False)

    # all partition-major so each load is ~128 large contiguous descriptors
    hsT4 = nc.declare_dram_parameter("hsT4", [NST, 128, 8, 512], F16, isOutput=False)
    wq = nc.declare_dram_parameter("wqT", [128, 8, 256], F16, isOutput=False)
    wk = nc.declare_dram_parameter("wkT", [128, 8, 64], F16, isOutput=False)
    wv = nc.declare_dram_parameter("wvT", [128, 8, 64], F16, isOutput=False)
    wo = nc.declare_dram_parameter("woT", [128, 2, D], F16, isOutput=False)
    cosd = nc.declare_dram_parameter("cos2", [128, S], F16, isOutput=False)
    sind = nc.declare_dram_parameter("sinS2", [128, S], F16, isOutput=False)
    maskd = nc.declare_dram_parameter("bandmask", [128, 2, 512], F16, isOutput=False)
    permd = nc.declare_dram_parameter("permT", [128, 128], F16, isOutput=False)
    o_part = nc.declare_dram_parameter("o_part", [S, D], F16, isOutput=True)

    with tile.TileContext(nc) as tc:
        with (
            tc.tile_pool(name="persist", bufs=1) as pp,
            tc.tile_pool(name="hts", bufs=4) as hpool,
            tc.tile_pool(name="work", bufs=2) as wkp,
            tc.tile_pool(name="expm", bufs=18) as ep,
            tc.tile_pool(name="norm", bufs=3) as npool,
            tc.tile_pool(name="ost", bufs=3) as opool,
            tc.tile_pool(name="pmm", bufs=4, space="PSUM") as pmm,
        ):
            # ---- persistent SBUF tiles ----
            warm = pp.tile([128, 512], F16, tag="warm")
            nc.vector.memset(warm, 0.0)

            wq_sb = pp.tile([128, 8, 256], F16, tag="wq")
            wk_sb = pp.tile([128, 8, 64], F16, tag="wk")
            wv_sb = pp.tile([128, 8, 64], F16, tag="wv")
            wo_sb = pp.tile([128, 2, D], F16, tag="wo")
            cos_sb = pp.tile([128, S], F16, tag="cos")
            sin_sb = pp.tile([128, S], F16, tag="sin")
            mask_sb = pp.tile([128, 2, 512], F16, tag="mask")
            perm_sb = pp.tile([128, 128], F16, tag="perm")

            qT = [pp.tile([128, S], F16, tag=f"qT{m}", name=f"qT{m}") for m in range(2)]
            kdup = pp.tile([128, S], F16, tag="kdup")
            vaug = pp.tile([128, NKT, 65], F16, tag="vaug")
            nc.vector.memset(vaug[:, :, 64:65], 1.0)
            ones64 = pp.tile([128, 64], F16, tag="ones64")
            nc.vector.memset(ones64, 1.0)
            aT = [pp.tile([128, S], F16, tag=f"aT{m}", name=f"aT{m}") for m in range(2)]

            # ---- input DMA stream (ordered on the sync queue) ----
            nc.sync.dma_start(out=wq_sb, in_=wq[:, :, :])
            nc.sync.dma_start(out=wk_sb, in_=wk[:, :, :])
            nc.sync.dma_start(out=wv_sb, in_=wv[:, :, :])
            nc.sync.dma_start(out=perm_sb, in_=permd[:, :])
            nc.sync.dma_start(out=mask_sb, in_=maskd[:, :, :])
            ht = []
            for s_ in range(NST):
                t = hpool.tile([128, 8, 512], F16, tag="ht", name=f"ht{s_}")
                nc.sync.dma_start(out=t, in_=hsT4[s_][:, :, :])
                ht.append(t)
                if s_ == 0:
                    nc.sync.dma_start(out=cos_sb, in_=cosd[:, :])
                    nc.sync.dma_start(out=sin_sb, in_=sind[:, :])
                if s_ == 1:
                    nc.sync.dma_start(out=wo_sb, in_=wo[:, :, :])

            # ---- PE warm-up (no DMA deps; ramps HAM while inputs stream) ----
            for _w in range(16):
                wmm = pmm.tile([128, 2, 512], F32, tag="mm", name=f"warm{_w}")
                nc.tensor.matmul(wmm[:, 0, :], warm[:, 0:128], warm,
                                 start=True, stop=True)

            expm = {}

            def proj_strip(s):
                sl = bass.ts(s, 512)
                # q projections (m = head-pair of the weight columns)
                qps = pmm.tile([128, 2, 512], F32, tag="mm", name=f"q{s}")
                raws = []
                for m in range(2):
                    for dt in range(8):
                        nc.tensor.matmul(qps[:, m, :], wq_sb[:, dt, bass.ts(m, 128)],
                                         ht[s][:, dt, :],
                                         start=(dt == 0), stop=(dt == 7))
                    raw = wkp.tile([128, 512], F16, tag=f"raw{m}")
                    nc.scalar.copy(raw, qps[:, m, :])
                    raws.append(raw)
                # k projection (64 hd rows) + v projection (direct [pos, hd])
                kvps = pmm.tile([128, 2, 512], F32, tag="mm", name=f"kv{s}")
                for dt in range(8):
                    nc.tensor.matmul(kvps[0:64, 0, :], wk_sb[:, dt, :],
                                     ht[s][:, dt, :],
                                     start=(dt == 0), stop=(dt == 7))
                rawk = wkp.tile([64, 512], F16, tag="rawk")
                nc.scalar.copy(rawk, kvps[0:64, 0, :])
                for pt in range(4):
                    for dt in range(8):
                        nc.tensor.matmul(kvps[:, 1, bass.ts(pt, 64)],
                                         ht[s][:, dt, bass.ts(pt, 128)],
                                         wv_sb[:, dt, :],
                                         start=(pt == 0 and dt == 0),
                                         stop=(pt == 3 and dt == 7),
                                         skip_group_check=True)
                nc.vector.tensor_copy(vaug[:, 4 * s:4 * s + 4, 0:64], kvps[:, 1, 0:256])
                # rope rotations + combines
                rot = pmm.tile([128, 2, 512], F32, tag="mm", name=f"rot{s}")
                for m in range(2):
                    nc.tensor.matmul(rot[:, m, :], perm_sb, raws[m],
                                     start=True, stop=True)
                rotk = pmm.tile([128, 2, 512], F32, tag="mm", name=f"rotk{s}")
                nc.tensor.matmul(rotk[0:64, 0, :], perm_sb[0:64, 0:64], rawk,
                                 start=True, stop=True)
                for m in range(2):
                    t1 = wkp.tile([128, 512], F16, tag=f"t1{m}")
                    nc.vector.tensor_tensor(out=t1, in0=rot[:, m, :],
                                            in1=sin_sb[:, sl], op=OP.mult)
                    t2 = wkp.tile([128, 512], F16, tag=f"t2{m}")
                    nc.gpsimd.tensor_tensor(out=t2, in0=raws[m], in1=cos_sb[:, sl],
                                            op=OP.mult)
                    nc.vector.tensor_tensor(out=qT[m][:, sl], in0=t1, in1=t2,
                                            op=OP.add)
                t1k = wkp.tile([64, 512], F16, tag="t1k")
                nc.vector.tensor_tensor(out=t1k, in0=rotk[0:64, 0, :],
                                        in1=sin_sb[0:64, sl], op=OP.mult)
                t2k = wkp.tile([64, 512], F16, tag="t2k")
                nc.gpsimd.tensor_tensor(out=t2k, in0=rawk, in1=cos_sb[0:64, sl],
                                        op=OP.mult)
                nc.vector.tensor_tensor(out=kdup[0:64, sl], in0=t1k, in1=t2k,
                                        op=OP.add)
                nc.gpsimd.dma_start(out=kdup[64:128, sl], in_=kdup[0:64, sl])

            def scores_part(s, part):
                """Scores + exp + band-mask for strip s's k-tiles.

                part 'A': window columns inside strip s (needs only rope(s));
                part 'B': spill-over columns in strip s+1 (needs rope(s+1)).
                The A/B column split at the strip boundary coincides exactly
                with the PV piece boundaries, so q-strip s's PV consumes only
                A-parts of strip s and B-parts of strip s-1 — all available
                one iteration before PV runs."""
                for kt in range(4 * s, 4 * s + 4):
                    j0 = 128 * kt
                    w = min(WIN, S - j0)
                    j = kt % 4
                    c0, c1 = (0, min(w, 512 - 128 * j)) if part == "A" else \
                             (min(w, 512 - 128 * j), w)
                    if c0 >= c1:
                        continue
                    for hp in range(2):
                        if part == "A":
                            pss = pmm.tile([128, 2, 512], F32, tag="mm",
                                           name=f"scA{kt}_{hp}")
                            et = ep.tile([128, 2, 512], F16, tag="e")
                            expm[(kt, hp)] = et
                        else:
                            pss = pmm.tile([128, 2, 512], F32, tag="mm",
                                           name=f"scB{kt}_{hp}")
                            et = expm[(kt, hp)]
                        for hh in range(2):
                            h = 2 * hp + hh
                            m, half = h // 2, (h % 2) * 64
                            nc.tensor.matmul(
                                pss[:, hh, c0:c1],
                                kdup[half:half + 64, bass.ts(kt, 128)],
                                qT[m][half:half + 64, j0 + c0:j0 + c1],
                                start=True, stop=True)
                        for hh in range(2):
                            nc.scalar.activation(et[:, hh, c0:c1], pss[:, hh, c0:c1],
                                                 AF.Exp, scale=float(SCALE))
                        # per-hh contiguous slices keep the DVE in 2x mode
                        nc.vector.tensor_tensor(out=et[:, 0, c0:c1],
                                                in0=et[:, 0, c0:c1],
                                                in1=mask_sb[:, 0, c0:c1],
                                                op=OP.mult)
                        nc.gpsimd.tensor_tensor(out=et[:, 1, c0:c1],
                                                in0=et[:, 1, c0:c1],
                                                in1=mask_sb[:, 1, c0:c1],
                                                op=OP.mult)

            def pv_norm(s):
                pieces = _pv_pieces(s)
                sl = bass.ts(s, 512)
                pvss = []
                for hp in range(2):
                    pvp = pmm.tile([128, 2, 512], F32, tag="mm", name=f"pv{s}_{hp}")
                    for i, (kt, c0, c1) in enumerate(pieces):
                        base = 128 * kt + c0 - 512 * s
                        for hh in range(2):
                            nc.tensor.matmul(pvp[0:65, hh, base:base + (c1 - c0)],
                                             vaug[:, kt, 0:65],
                                             expm[(kt, hp)][:, hh, c0:c1],
                                             start=(i == 0), stop=(i == len(pieces) - 1),
                                             skip_group_check=True)
                    # evacuate psum immediately; the slot frees in <1us
                    pvs = npool.tile([65, 2, 512], F16, tag=f"pvs{hp}")
                    nc.vector.tensor_copy(pvs, pvp[0:65, :, :])
                    pvss.append(pvs)
                for hp in range(2):
                    pvs = pvss[hp]
                    # denominators (row 64, from the ones column of vaug) are
                    # broadcast to 64 partitions by a K=1 ones matmul, then
                    # reciprocal'd on-chip — no DMA hops on this chain
                    bcp = pmm.tile([128, 2, 512], F32, tag="mm", name=f"bc{s}_{hp}")
                    for hh in range(2):
                        nc.tensor.matmul(bcp[0:64, hh, :], ones64[64:65, :],
                                         pvs[64:65, hh, :], start=True, stop=True)
                    rbc = npool.tile([64, 2, 512], F32, tag=f"rbc{hp}")
                    nc.vector.reciprocal_approx_fast(out=rbc, in_=bcp[0:64, :, :])
                    nc.vector.tensor_tensor(out=aT[hp][0:64, sl],
                                            in0=pvs[0:64, 0, :], in1=rbc[:, 0, :],
                                            op=OP.mult)
                    stag = npool.tile([64, 512], F16, tag="stag")
                    nc.vector.tensor_tensor(out=stag, in0=pvs[0:64, 1, :],
                                            in1=rbc[:, 1, :], op=OP.mult)
                    nc.scalar.dma_start(out=aT[hp][64:128, sl], in_=stag)

            def oproj(s):
                for j in range(4):
                    st = 4 * s + j
                    psos = pmm.tile([128, 2, 512], F32, tag="mm", name=f"o{st}")
                    for ch in range(2):
                        for dsp in range(2):
                            nc.tensor.matmul(psos[:, dsp, :],
                                             aT[ch][:, bass.ts(st, 128)],
                                             wo_sb[:, ch, bass.ts(dsp, 512)],
                                             start=(ch == 0), stop=(ch == 1))
                    ost = opool.tile([128, 1024], F16, tag="o")
                    nc.scalar.copy(ost[:, 0:512], psos[:, 0, :])
                    nc.vector.tensor_copy(ost[:, 512:1024], psos[:, 1, :])
                    nc.sync.dma_start(out=o_part[bass.ts(st, 128), :], in_=ost)

            for i in range(5):
                if i <= 3:
                    proj_strip(i)
                if i >= 1:
                    pv_norm(i - 1)      # inputs complete since iteration i-1
                if i <= 3:
                    scores_part(i, "A")
                if 1 <= i <= 3:
                    scores_part(i - 1, "B")
                if i >= 1:
                    oproj(i - 1)

    nc.compile()
    return nc


_NC = {}


def _get_nc():
    if "k" not in _NC:
        _NC["k"] = _build()
    return _NC["k"]


def _host_inputs(hidden_states, position_ids, Wq, Wk, Wv, Wo):
    hs = np.asarray(hidden_states, np.float32)
    Wq = np.asarray(Wq, np.float32)
    Wk = np.asarray(Wk, np.float32)
    Wv = np.asarray(Wv, np.float32)
    Wo = np.asarray(Wo, np.float32)

    hsT4 = []
    for b in range(B):
        hT = np.ascontiguousarray(hs[b].T).astype(np.float16)      # [D, S]
        # [NST, 128, 8, 512]: strip-major, partition-major within strip
        h4 = np.empty((NST, 128, 8, 512), np.float16)
        for s_ in range(NST):
            for dt in range(8):
                h4[s_, :, dt] = hT[128 * dt:128 * (dt + 1), 512 * s_:512 * (s_ + 1)]
        hsT4.append(np.ascontiguousarray(h4))

    inv_freq = (1.0 / (THETA ** (np.arange(0, HD, 2, dtype=np.float32) / HD))).astype(np.float32)
    cos2, sin2 = [], []
    for b in range(B):
        pos = np.asarray(position_ids[b]).astype(np.float32)
        freqs = pos[:, None] * inv_freq[None, :]          # [S, 32]
        cosf = np.cos(freqs).T                            # [32, S]
        sinf = np.sin(freqs).T
        cos64 = np.concatenate([cosf, cosf], axis=0)      # [64, S]
        sin64s = np.concatenate([-sinf, sinf], axis=0)    # sign-folded
        cos2.append(np.concatenate([cos64, cos64], axis=0).astype(np.float16))
        sin2.append(np.concatenate([sin64s, sin64s], axis=0).astype(np.float16))

    p = np.arange(128)[:, None]
    c = np.arange(WIN)[None, :]
    band = ((p <= c) & (c < p + WINDOW)).astype(np.float16)   # [128, 384]
    mask2 = np.zeros((128, 2, 512), np.float16)
    mask2[:, 0, 0:WIN] = band
    mask2[:, 1, 0:WIN] = band

    perm = np.zeros((64, 64), np.float32)
    for i in range(32):
        perm[i, i + 32] = 1.0
        perm[i + 32, i] = 1.0
    perm2 = np.kron(np.eye(2, dtype=np.float32), perm)    # [128, 128]
    permT = np.ascontiguousarray(perm2.T).astype(np.float16)

    in_maps = []
    for core in range(8):
        b, g = divmod(core, 4)
        # partition-major: [128, n_chunks, cols]
        wqT = np.ascontiguousarray(
            Wq[256 * g:256 * (g + 1)].T.astype(np.float16)
            .reshape(8, 128, 256).transpose(1, 0, 2))
        wkT = np.ascontiguousarray(
            Wk[64 * g:64 * (g + 1)].T.astype(np.float16)
            .reshape(8, 128, 64).transpose(1, 0, 2))
        wvT = np.ascontiguousarray(
            Wv[64 * g:64 * (g + 1)].T.astype(np.float16)
            .reshape(8, 128, 64).transpose(1, 0, 2))
        woT = np.ascontiguousarray(
            Wo[:, 256 * g:256 * (g + 1)].T.astype(np.float16)
            .reshape(2, 128, D).transpose(1, 0, 2))
        in_maps.append({
            "hsT4": hsT4[b], "wqT": wqT, "wkT": wkT, "wvT": wvT, "woT": woT,
            "cos2": cos2[b], "sinS2": sin2[b],
            "bandmask": mask2, "permT": permT,
        })
    return in_maps


def run_spmd(hidden_states, attention_mask, position_ids, Wq, Wk, Wv, Wo, **spmd_kwargs):
    nc = _get_nc()
    in_maps = _host_inputs(hidden_states, position_ids, Wq, Wk, Wv, Wo)
    res = run_bass_kernel_spmd(nc, in_maps, list(range(8)), **spmd_kwargs)
    out = np.zeros((B, S, D), np.float32)
    for core in range(8):
        out[core // 4] += np.asarray(res.results[core]["o_part"], np.float32)
    return out, res


def kernel(hidden_states, attention_mask, position_ids, Wq, Wk, Wv, Wo):
    out, _ = run_spmd(hidden_states, attention_mask, position_ids, Wq, Wk, Wv, Wo)
    return out


# revision 27
# speedup vs baseline: 1.5342x; 1.0694x over previous
"""Mimi-style GQA attention (RoPE + 250-wide sliding causal window) on 8 TRN2 NeuronCores.

Sharding: core c handles batch b=c//4 and KV-head group g=c%4 (4 query heads +
1 KV head). Wq/Wk/Wv column-sharded, Wo row-sharded along the head dim; each
core emits a partial [S, D] f16 output; host sums the 4 partials per batch.

Strip-streamed pipeline (4 q-strips of 512, software-pipelined over 6
iterations) keeps the PE matmul queue dense so the HAM clock gate stays open:
  iter i: proj+rope strip i | scores for strip i-1's k-tiles | PV+norm for
  q-strip i-1 | output projection for strip i-2.
Implementation notes:
  - hs streamed per strip; projections start as soon as strip 0 lands.
  - One rotating PSUM pool of 4x [128,2,512] 2-bank slots (8 banks total)
    serves q-proj, k+v, rope-rot, score pairs, PV and O-proj tiles; pairing
    two heads per slot halves the PE->ACT handoff count.
  - V is projected directly in [pos, hd] orientation (hs chunk stationary),
    so no PE transposes; a ones column in vaug yields softmax denominators
    for free in PV row 64.
  - PV accumulation needs no zero-seed matmuls: pieces ascend in kt, so the
    first piece (start=True) clears the bank and later start=False matmuls
    accumulate where has_written is set and overwrite fresh columns.
  - PV psum is evacuated to SBUF (f16) immediately, freeing the slot in
    <1us; the softmax-normalization chain (DMA reshape -> DVE
    reciprocal_approx_fast -> DMA -> gpsimd partition broadcast -> DVE
    multiplies) runs off the SBUF copy one iteration ahead of the O-proj.
"""
import os
import sys

for _p in ("/opt/trn_rl_repo", "/root/.axon_site/_ro/trn_rl_repo"):
    if os.path.isdir(_p) and _p not in sys.path:
        sys.path.append(_p)

import numpy as np
import concourse.bass as bass
import concourse.mybir as mybir
import concourse.tile as tile
from concourse import bacc
from concourse.bass_utils import run_bass_kernel_spmd

F32 = mybir.dt.float32
F16 = mybir.dt.float16
AF = mybir.ActivationFunctionType
OP = mybir.AluOpType

B, S, D = 2, 2048, 1024
H, HK, HD = 16, 4, 64
WINDOW = 250
SCALE = 1.0 / np.sqrt(HD)
THETA = 10000.0
NKT = S // 128          # 16 k-tiles
NST = S // 512          # 4 q-strips
WIN = 384               # padded per-k-tile q-window


def _pv_pieces(s):
    """PV pieces for q-strip s, ascending kt: list of (kt, c0, c1) ranges."""
    out = []
    for kt in range(max(0, 4 * s - 2), min(NKT - 1, 4 * s + 3) + 1):
        j0 = 128 * kt
        w = min(WIN, S - j0)
        c_lo = max(0, 512 * s - j0)
        c_hi = min(w, 512 * (s + 1) - j0)
        if c_lo < c_hi:
            out.append((kt, c_lo, c_hi))
    return out


def _build():
    nc = bacc.Bacc(None, target_bir_lowering=False)

    # all partition-major so each load is ~128 large contiguous descriptors
    hsT4 = nc.declare_dram_parameter("hsT4", [NST, 128, 8, 512], F16, isOutput=False)
    wq = nc.declare_dram_parameter("wqT", [128, 8, 256], F16, isOutput=False)
    wk = nc.declare_dram_parameter("wkT", [128, 8, 64], F16, isOutput=False)
    wv = nc.declare_dram_parameter("wvT", [128, 8, 64], F16, isOutput=False)
    wo = nc.declare_dram_parameter("woT", [128, 2, D], F16, isOutput=False)
    cosd = nc.declare_dram_parameter("cos2", [128, S], F16, isOutput=False)
    sind = nc.declare_dram_parameter("sinS2", [128, S], F16, isOutput=False)
    maskd = nc.declare_dram_parameter("bandmask", [128, 2, 512], F16, isOutput=False)
    permd = nc.declare_dram_parameter("permT", [128, 128], F16, isOutput=False)
    o_part = nc.declare_dram_parameter("o_part", [S, D], F16, isOutput=True)

    with tile.TileContext(nc) as tc:
        with (
            tc.tile_pool(name="persist", bufs=1) as pp,
            tc.tile_pool(name="hts", bufs=4) as hpool,
            tc.tile_pool(name="work", bufs=2) as wkp,
            tc.tile_pool(name="expm", bufs=18) as ep,
            tc.tile_pool(name="norm", bufs=3) as npool,
            tc.tile_pool(name="ost", bufs=3) as opool,
            tc.tile_pool(name="pmm", bufs=4, space="PSUM") as pmm,
        ):
            # ---- persistent SBUF tiles ----
            warm = pp.tile([128, 512], F16, tag="warm")
            nc.vector.memset(warm, 0.0)

            wq_sb = pp.tile([128, 8, 256], F16, tag="wq")
            wk_sb = pp.tile([128, 8, 64], F16, tag="wk")
            wv_sb = pp.tile([128, 8, 64], F16, tag="wv")
            wo_sb = pp.tile([128, 2, D], F16, tag="wo")
            cos_sb = pp.tile([128, S], F16, tag="cos")
            sin_sb = pp.tile([128, S], F16, tag="sin")
            mask_sb = pp.tile([128, 2, 512], F16, tag="mask")
            perm_sb = pp.tile([128, 128], F16, tag="perm")

            qT = [pp.tile([128, S], F16, tag=f"qT{m}", name=f"qT{m}") for m in range(2)]
            kdup = pp.tile([128, S], F16, tag="kdup")
            vaug = pp.tile([128, NKT, 65], F16, tag="vaug")
            nc.vector.memset(vaug[:, :, 64:65], 1.0)
            ones64 = pp.tile([128, 64], F16, tag="ones64")
            nc.vector.memset(ones64, 1.0)
            aT = [pp.tile([128, S], F16, tag=f"aT{m}", name=f"aT{m}") for m in range(2)]

            # ---- input DMA stream (ordered on the sync queue) ----
            ht = [hpool.tile([128, 8, 512], F16, tag="ht", name=f"ht{s_}")
                  for s_ in range(NST)]
            nc.sync.dma_start(out=wq_sb, in_=wq[:, :, :])
            nc.sync.dma_start(out=ht[0], in_=hsT4[0][:, :, :])
            nc.sync.dma_start(out=wk_sb, in_=wk[:, :, :])
            nc.sync.dma_start(out=wv_sb, in_=wv[:, :, :])
            nc.sync.dma_start(out=perm_sb, in_=permd[:, :])
            nc.sync.dma_start(out=mask_sb, in_=maskd[:, :, :])
            nc.sync.dma_start(out=cos_sb, in_=cosd[:, :])
            nc.sync.dma_start(out=sin_sb, in_=sind[:, :])
            nc.sync.dma_start(out=ht[1], in_=hsT4[1][:, :, :])
            nc.sync.dma_start(out=wo_sb, in_=wo[:, :, :])
            nc.sync.dma_start(out=ht[2], in_=hsT4[2][:, :, :])
            nc.sync.dma_start(out=ht[3], in_=hsT4[3][:, :, :])

            # ---- PE warm-up (no DMA deps; ramps HAM while inputs stream) ----
            for _w in range(12):
                wmm = pmm.tile([128, 2, 512], F32, tag="mm", name=f"warm{_w}")
                nc.tensor.matmul(wmm[:, 0, :], warm[:, 0:128], warm,
                                 start=True, stop=True)

            expm = {}

            def proj_strip(s):
                sl = bass.ts(s, 512)
                # q projections (m = head-pair of the weight columns)
                qps = pmm.tile([128, 2, 512], F32, tag="mm", name=f"q{s}")
                raws = []
                for m in range(2):
                    for dt in range(8):
                        nc.tensor.matmul(qps[:, m, :], wq_sb[:, dt, bass.ts(m, 128)],
                                         ht[s][:, dt, :],
                                         start=(dt == 0), stop=(dt == 7))
                    raw = wkp.tile([128, 512], F16, tag=f"raw{m}")
                    nc.scalar.copy(raw, qps[:, m, :])
                    raws.append(raw)
                # k projection (64 hd rows) + v projection (direct [pos, hd])
                kvps = pmm.tile([128, 2, 512], F32, tag="mm", name=f"kv{s}")
                for dt in range(8):
                    nc.tensor.matmul(kvps[0:64, 0, :], wk_sb[:, dt, :],
                                     ht[s][:, dt, :],
                                     start=(dt == 0), stop=(dt == 7))
                rawk = wkp.tile([64, 512], F16, tag="rawk")
                nc.scalar.copy(rawk, kvps[0:64, 0, :])
                for pt in range(4):
                    for dt in range(8):
                        nc.tensor.matmul(kvps[:, 1, bass.ts(pt, 64)],
                                         ht[s][:, dt, bass.ts(pt, 128)],
                                         wv_sb[:, dt, :],
                                         start=(pt == 0 and dt == 0),
                                         stop=(pt == 3 and dt == 7),
                                         skip_group_check=True)
                nc.scalar.copy(vaug[:, 4 * s:4 * s + 4, 0:64], kvps[:, 1, 0:256])
                # rope rotations + combines
                rot = pmm.tile([128, 2, 512], F32, tag="mm", name=f"rot{s}")
                for m in range(2):
                    nc.tensor.matmul(rot[:, m, :], perm_sb, raws[m],
                                     start=True, stop=True)
                rotk = pmm.tile([128, 2, 512], F32, tag="mm", name=f"rotk{s}")
                nc.tensor.matmul(rotk[0:64, 0, :], perm_sb[0:64, 0:64], rawk,
                                 start=True, stop=True)
                for m in range(2):
                    t1 = wkp.tile([128, 512], F16, tag=f"t1{m}")
                    nc.vector.tensor_tensor(out=t1, in0=rot[:, m, :],
                                            in1=sin_sb[:, sl], op=OP.mult)
                    t2 = wkp.tile([128, 512], F16, tag=f"t2{m}")
                    nc.gpsimd.tensor_tensor(out=t2, in0=raws[m], in1=cos_sb[:, sl],
                                            op=OP.mult)
                    nc.vector.tensor_tensor(out=qT[m][:, sl], in0=t1, in1=t2,
                                            op=OP.add)
                t1k = wkp.tile([64, 512], F16, tag="t1k")
                nc.vector.tensor_tensor(out=t1k, in0=rotk[0:64, 0, :],
                                        in1=sin_sb[0:64, sl], op=OP.mult)
                t2k = wkp.tile([64, 512], F16, tag="t2k")
                nc.gpsimd.tensor_tensor(out=t2k, in0=rawk, in1=cos_sb[0:64, sl],
                                        op=OP.mult)
                nc.vector.tensor_tensor(out=kdup[0:64, sl], in0=t1k, in1=t2k,
                                        op=OP.add)
                nc.gpsimd.dma_start(out=kdup[64:128, sl], in_=kdup[0:64, sl])

            def scores_part(s, part):
                """Scores + exp + band-mask for strip s's k-tiles.

                part 'A': window columns inside strip s (needs only rope(s));
                part 'B': spill-over columns in strip s+1 (needs rope(s+1)).
                The A/B column split at the strip boundary coincides exactly
                with the PV piece boundaries, so q-strip s's PV consumes only
                A-parts of strip s and B-parts of strip s-1 — all available
                one iteration before PV runs."""
                for kt in range(4 * s, 4 * s + 4):
                    j0 = 128 * kt
                    w = min(WIN, S - j0)
                    j = kt % 4
                    c0, c1 = (0, min(w, 512 - 128 * j)) if part == "A" else \
                             (min(w, 512 - 128 * j), w)
                    if c0 >= c1:
                        continue
                    for hp in range(2):
                        if part == "A":
                            pss = pmm.tile([128, 2, 512], F32, tag="mm",
                                           name=f"scA{kt}_{hp}")
                            et = ep.tile([128, 2, 512], F16, tag="e")
                            expm[(kt, hp)] = et
                        else:
                            pss = pmm.tile([128, 2, 512], F32, tag="mm",
                                           name=f"scB{kt}_{hp}")
                            et = expm[(kt, hp)]
                        for hh in range(2):
                            h = 2 * hp + hh
                            m, half = h // 2, (h % 2) * 64
                            nc.tensor.matmul(
                                pss[:, hh, c0:c1],
                                kdup[half:half + 64, bass.ts(kt, 128)],
                                qT[m][half:half + 64, j0 + c0:j0 + c1],
                                start=True, stop=True)
                        nc.scalar.activation(et[:, :, c0:c1], pss[:, :, c0:c1],
                                             AF.Exp, scale=float(SCALE))
                        # per-hh contiguous slices keep the DVE in 2x mode
                        nc.vector.tensor_tensor(out=et[:, 0, c0:c1],
                                                in0=et[:, 0, c0:c1],
                                                in1=mask_sb[:, 0, c0:c1],
                                                op=OP.mult)
                        nc.gpsimd.tensor_tensor(out=et[:, 1, c0:c1],
                                                in0=et[:, 1, c0:c1],
                                                in1=mask_sb[:, 1, c0:c1],
                                                op=OP.mult)

            def pv_norm(s):
                pieces = _pv_pieces(s)
                sl = bass.ts(s, 512)
                pvss = []
                for hp in range(2):
                    pvp = pmm.tile([128, 2, 512], F32, tag="mm", name=f"pv{s}_{hp}")
                    for i, (kt, c0, c1) in enumerate(pieces):
                        base = 128 * kt + c0 - 512 * s
                        for hh in range(2):
                            nc.tensor.matmul(pvp[0:65, hh, base:base + (c1 - c0)],
                                             vaug[:, kt, 0:65],
                                             expm[(kt, hp)][:, hh, c0:c1],
                                             start=(i == 0), stop=(i == len(pieces) - 1),
                                             skip_group_check=True)
                    # evacuate psum immediately; the slot frees in <1us
                    pvs = npool.tile([65, 2, 512], F16, tag=f"pvs{hp}")
                    nc.scalar.copy(pvs, pvp[0:65, :, :])
                    pvss.append(pvs)
                for hp in range(2):
                    pvs = pvss[hp]
                    # denominators (row 64, from the ones column of vaug) are
                    # broadcast to 64 partitions by a K=1 ones matmul, then
                    # reciprocal'd on-chip — no DMA hops on this chain
                    bcp = pmm.tile([128, 2, 512], F32, tag="mm", name=f"bc{s}_{hp}")
                    for hh in range(2):
                        nc.tensor.matmul(bcp[0:64, hh, :], ones64[64:65, :],
                                         pvs[64:65, hh, :], start=True, stop=True)
                    rbc = npool.tile([64, 2, 512], F32, tag=f"rbc{hp}")
                    nc.vector.reciprocal_approx_fast(out=rbc, in_=bcp[0:64, :, :])
                    nc.vector.tensor_tensor(out=aT[hp][0:64, sl],
                                            in0=pvs[0:64, 0, :], in1=rbc[:, 0, :],
                                            op=OP.mult)
                    stag = npool.tile([64, 512], F16, tag="stag")
                    nc.vector.tensor_tensor(out=stag, in0=pvs[0:64, 1, :],
                                            in1=rbc[:, 1, :], op=OP.mult)
                    nc.scalar.dma_start(out=aT[hp][64:128, sl], in_=stag)

            def oproj(s):
                for j in range(4):
                    st = 4 * s + j
                    psos = pmm.tile([128, 2, 512], F32, tag="mm", name=f"o{st}")
                    for ch in range(2):
                        for dsp in range(2):
                            nc.tensor.matmul(psos[:, dsp, :],
                                             aT[ch][:, bass.ts(st, 128)],
                                             wo_sb[:, ch, bass.ts(dsp, 512)],
                                             start=(ch == 0), stop=(ch == 1))
                    ost = opool.tile([128, 1024], F16, tag="o")
                    if j < 2:
                        nc.scalar.copy(ost, psos[:, :, :])
                    else:
                        nc.vector.tensor_copy(ost, psos[:, :, :])
                    nc.sync.dma_start(out=o_part[bass.ts(st, 128), :], in_=ost)

            for i in range(5):
                if i <= 3:
                    proj_strip(i)
                if i >= 1:
                    pv_norm(i - 1)      # inputs complete since iteration i-1
                if i <= 3:
                    scores_part(i, "A")
                if 1 <= i <= 3:
                    scores_part(i - 1, "B")
                if i >= 1:
                    oproj(i - 1)

    nc.compile()
    return nc


_NC = {}


def _get_nc():
    if "k" not in _NC:
        _NC["k"] = _build()
    return _NC["k"]


def _host_inputs(hidden_states, position_ids, Wq, Wk, Wv, Wo):
    hs = np.asarray(hidden_states, np.float32)
    Wq = np.asarray(Wq, np.float32)
    Wk = np.asarray(Wk, np.float32)
    Wv = np.asarray(Wv, np.float32)
    Wo = np.asarray(Wo, np.float32)

    hsT4 = []
    for b in range(B):
        hT = np.ascontiguousarray(hs[b].T).astype(np.float16)      # [D, S]
        # [NST, 128, 8, 512]: strip-major, partition-major within strip
        h4 = np.empty((NST, 128, 8, 512), np.float16)
        for s_ in range(NST):
            for dt in range(8):
                h4[s_, :, dt] = hT[128 * dt:128 * (dt + 1), 512 * s_:512 * (s_ + 1)]
        hsT4.append(np.ascontiguousarray(h4))

    inv_freq = (1.0 / (THETA ** (np.arange(0, HD, 2, dtype=np.float32) / HD))).astype(np.float32)
    cos2, sin2 = [], []
    for b in range(B):
        pos = np.asarray(position_ids[b]).astype(np.float32)
        freqs = pos[:, None] * inv_freq[None, :]          # [S, 32]
        cosf = np.cos(freqs).T                            # [32, S]
        sinf = np.sin(freqs).T
        cos64 = np.concatenate([cosf, cosf], axis=0)      # [64, S]
        sin64s = np.concatenate([-sinf, sinf], axis=0)    # sign-folded
        cos2.append(np.concatenate([cos64, cos64], axis=0).astype(np.float16))
        sin2.append(np.concatenate([sin64s, sin64s], axis=0).astype(np.float16))

    p = np.arange(128)[:, None]
    c = np.arange(WIN)[None, :]
    band = ((p <= c) & (c < p + WINDOW)).astype(np.float16)   # [128, 384]
    mask2 = np.zeros((128, 2, 512), np.float16)
    mask2[:, 0, 0:WIN] = band
    mask2[:, 1, 0:WIN] = band

    perm = np.zeros((64, 64), np.float32)
    for i in range(32):
        perm[i, i + 32] = 1.0
        perm[i + 32, i] = 1.0
    perm2 = np.kron(np.eye(2, dtype=np.float32), perm)    # [128, 128]
    permT = np.ascontiguousarray(perm2.T).astype(np.float16)

    in_maps = []
    for core in range(8):
        b, g = divmod(core, 4)
        # partition-major: [128, n_chunks, cols]
        wqT = np.ascontiguousarray(
            Wq[256 * g:256 * (g + 1)].T.astype(np.float16)
            .reshape(8, 128, 256).transpose(1, 0, 2))
        wkT = np.ascontiguousarray(
            Wk[64 * g:64 * (g + 1)].T.astype(np.float16)
            .reshape(8, 128, 64).transpose(1, 0, 2))
        wvT = np.ascontiguousarray(
            Wv[64 * g:64 * (g + 1)].T.astype(np.float16)
            .reshape(8, 128, 64).transpose(1, 0, 2))
        woT = np.ascontiguousarray(
            Wo[:, 256 * g:256 * (g + 1)].T.astype(np.float16)
            .reshape(2, 128, D).transpose(1, 0, 2))
        in_maps.append({
            "hsT4": hsT4[b], "wqT": wqT, "wkT": wkT, "wvT": wvT, "woT": woT,
            "cos2": cos2[b], "sinS2": sin2[b],
            "bandmask": mask2, "permT": permT,
        })
    return in_maps


def run_spmd(hidden_states, attention_mask, position_ids, Wq, Wk, Wv, Wo, **spmd_kwargs):
    nc = _get_nc()
    in_maps = _host_inputs(hidden_states, position_ids, Wq, Wk, Wv, Wo)
    res = run_bass_kernel_spmd(nc, in_maps, list(range(8)), **spmd_kwargs)
    out = np.zeros((B, S, D), np.float32)
    for core in range(8):
        out[core // 4] += np.asarray(res.results[core]["o_part"], np.float32)
    return out, res


def kernel(hidden_states, attention_mask, position_ids, Wq, Wk, Wv, Wo):
    out, _ = run_spmd(hidden_states, attention_mask, position_ids, Wq, Wk, Wv, Wo)
    return out


# revision 28
# speedup vs baseline: 1.5986x; 1.0420x over previous
"""Mimi-style GQA attention (RoPE + 250-wide sliding causal window) on 8 TRN2 NeuronCores.

Sharding: core c handles batch b=c//4 and KV-head group g=c%4 (4 query heads +
1 KV head). Wq/Wk/Wv column-sharded, Wo row-sharded along the head dim; each
core emits a partial [S, D] f16 output; host sums the 4 partials per batch.

Strip-streamed pipeline (4 q-strips of 512, software-pipelined over 6
iterations) keeps the PE matmul queue dense so the HAM clock gate stays open:
  iter i: proj+rope strip i | scores for strip i-1's k-tiles | PV+norm for
  q-strip i-1 | output projection for strip i-2.
Implementation notes:
  - hs streamed per strip; projections start as soon as strip 0 lands.
  - One rotating PSUM pool of 4x [128,2,512] 2-bank slots (8 banks total)
    serves q-proj, k+v, rope-rot, score pairs, PV and O-proj tiles; pairing
    two heads per slot halves the PE->ACT handoff count.
  - V is projected directly in [pos, hd] orientation (hs chunk stationary),
    so no PE transposes; a ones column in vaug yields softmax denominators
    for free in PV row 64.
  - PV accumulation needs no zero-seed matmuls: pieces ascend in kt, so the
    first piece (start=True) clears the bank and later start=False matmuls
    accumulate where has_written is set and overwrite fresh columns.
  - PV psum is evacuated to SBUF (f16) immediately, freeing the slot in
    <1us; the softmax-normalization chain (DMA reshape -> DVE
    reciprocal_approx_fast -> DMA -> gpsimd partition broadcast -> DVE
    multiplies) runs off the SBUF copy one iteration ahead of the O-proj.
"""
import os
import sys

for _p in ("/opt/trn_rl_repo", "/root/.axon_site/_ro/trn_rl_repo"):
    if os.path.isdir(_p) and _p not in sys.path:
        sys.path.append(_p)

import numpy as np
import concourse.bass as bass
import concourse.mybir as mybir
import concourse.tile as tile
from concourse import bacc
from concourse.bass_utils import run_bass_kernel_spmd

F32 = mybir.dt.float32
F16 = mybir.dt.float16
AF = mybir.ActivationFunctionType
OP = mybir.AluOpType

B, S, D = 2, 2048, 1024
H, HK, HD = 16, 4, 64
WINDOW = 250
SCALE = 1.0 / np.sqrt(HD)
THETA = 10000.0
NKT = S // 128          # 16 k-tiles
NST = S // 512          # 4 q-strips
WIN = 384               # padded per-k-tile q-window


def _pv_pieces(s):
    """PV pieces for q-strip s, ascending kt: list of (kt, c0, c1) ranges."""
    out = []
    for kt in range(max(0, 4 * s - 2), min(NKT - 1, 4 * s + 3) + 1):
        j0 = 128 * kt
        w = min(WIN, S - j0)
        c_lo = max(0, 512 * s - j0)
        c_hi = min(w, 512 * (s + 1) - j0)
        if c_lo < c_hi:
            out.append((kt, c_lo, c_hi))
    return out


def _build():
    nc = bacc.Bacc(None, target_bir_lowering=False)

    # all partition-major so each load is ~128 large contiguous descriptors
    hsT4 = nc.declare_dram_parameter("hsT4", [NST, 128, 8, 512], F16, isOutput=False)
    wq = nc.declare_dram_parameter("wqT", [128, 8, 256], F16, isOutput=False)
    wk = nc.declare_dram_parameter("wkT", [128, 8, 64], F16, isOutput=False)
    wv = nc.declare_dram_parameter("wvT", [128, 8, 64], F16, isOutput=False)
    wo = nc.declare_dram_parameter("woT", [128, 2, D], F16, isOutput=False)
    cosd = nc.declare_dram_parameter("cos2", [128, S], F16, isOutput=False)
    sind = nc.declare_dram_parameter("sinS2", [128, S], F16, isOutput=False)
    maskd = nc.declare_dram_parameter("bandmask", [128, 2, 512], F16, isOutput=False)
    permd = nc.declare_dram_parameter("permT", [128, 128], F16, isOutput=False)
    o_part = nc.declare_dram_parameter("o_part", [S, D], F16, isOutput=True)

    with tile.TileContext(nc) as tc:
        with (
            tc.tile_pool(name="persist", bufs=1) as pp,
            tc.tile_pool(name="hts", bufs=4) as hpool,
            tc.tile_pool(name="work", bufs=2) as wkp,
            tc.tile_pool(name="expm", bufs=18) as ep,
            tc.tile_pool(name="norm", bufs=3) as npool,
            tc.tile_pool(name="ost", bufs=3) as opool,
            tc.tile_pool(name="pmm", bufs=4, space="PSUM") as pmm,
        ):
            # ---- persistent SBUF tiles ----
            warm = pp.tile([128, 512], F16, tag="warm")
            nc.vector.memset(warm, 0.0)

            wq_sb = pp.tile([128, 8, 256], F16, tag="wq")
            wk_sb = pp.tile([128, 8, 64], F16, tag="wk")
            wv_sb = pp.tile([128, 8, 64], F16, tag="wv")
            wo_sb = pp.tile([128, 2, D], F16, tag="wo")
            cos_sb = pp.tile([128, S], F16, tag="cos")
            sin_sb = pp.tile([128, S], F16, tag="sin")
            mask_sb = pp.tile([128, 2, 512], F16, tag="mask")
            perm_sb = pp.tile([128, 128], F16, tag="perm")

            qT = [pp.tile([128, S], F16, tag=f"qT{m}", name=f"qT{m}") for m in range(2)]
            kdup = pp.tile([128, S], F16, tag="kdup")
            vaug = pp.tile([128, NKT, 65], F16, tag="vaug")
            nc.vector.memset(vaug[:, :, 64:65], 1.0)
            ones64 = pp.tile([128, 64], F16, tag="ones64")
            nc.vector.memset(ones64, 1.0)
            aT = [pp.tile([128, S], F16, tag=f"aT{m}", name=f"aT{m}") for m in range(2)]

            # ---- input DMA stream (ordered on the sync queue) ----
            ht = [hpool.tile([128, 8, 512], F16, tag="ht", name=f"ht{s_}")
                  for s_ in range(NST)]
            nc.sync.dma_start(out=wq_sb, in_=wq[:, :, :])
            nc.sync.dma_start(out=ht[0], in_=hsT4[0][:, :, :])
            nc.sync.dma_start(out=wk_sb, in_=wk[:, :, :])
            nc.sync.dma_start(out=wv_sb, in_=wv[:, :, :])
            nc.sync.dma_start(out=perm_sb, in_=permd[:, :])
            nc.sync.dma_start(out=mask_sb, in_=maskd[:, :, :])
            nc.sync.dma_start(out=cos_sb, in_=cosd[:, :])
            nc.sync.dma_start(out=sin_sb, in_=sind[:, :])
            nc.sync.dma_start(out=ht[1], in_=hsT4[1][:, :, :])
            nc.sync.dma_start(out=wo_sb, in_=wo[:, :, :])
            nc.sync.dma_start(out=ht[2], in_=hsT4[2][:, :, :])
            nc.sync.dma_start(out=ht[3], in_=hsT4[3][:, :, :])

            # ---- PE warm-up (no DMA deps; ramps HAM while inputs stream) ----
            for _w in range(12):
                wmm = pmm.tile([128, 2, 512], F32, tag="mm", name=f"warm{_w}")
                nc.tensor.matmul(wmm[:, 0, :], warm[:, 0:128], warm,
                                 start=True, stop=True)

            expm = {}

            def proj_strip(s):
                sl = bass.ts(s, 512)
                # q projections (m = head-pair of the weight columns)
                qps = pmm.tile([128, 2, 512], F32, tag="mm", name=f"q{s}")
                raws = []
                for m in range(2):
                    for dt in range(8):
                        nc.tensor.matmul(qps[:, m, :], wq_sb[:, dt, bass.ts(m, 128)],
                                         ht[s][:, dt, :],
                                         start=(dt == 0), stop=(dt == 7))
                    raw = wkp.tile([128, 512], F16, tag=f"raw{m}")
                    nc.scalar.copy(raw, qps[:, m, :])
                    raws.append(raw)
                # k projection (64 hd rows) + v projection (direct [pos, hd])
                kvps = pmm.tile([128, 2, 512], F32, tag="mm", name=f"kv{s}")
                for dt in range(8):
                    nc.tensor.matmul(kvps[0:64, 0, :], wk_sb[:, dt, :],
                                     ht[s][:, dt, :],
                                     start=(dt == 0), stop=(dt == 7))
                rawk = wkp.tile([64, 512], F16, tag="rawk")
                nc.scalar.copy(rawk, kvps[0:64, 0, :])
                for pt in range(4):
                    for dt in range(8):
                        nc.tensor.matmul(kvps[:, 1, bass.ts(pt, 64)],
                                         ht[s][:, dt, bass.ts(pt, 128)],
                                         wv_sb[:, dt, :],
                                         start=(pt == 0 and dt == 0),
                                         stop=(pt == 3 and dt == 7),
                                         skip_group_check=True)
                nc.scalar.copy(vaug[:, 4 * s:4 * s + 4, 0:64], kvps[:, 1, 0:256])
                # rope rotations + combines
                rot = pmm.tile([128, 2, 512], F32, tag="mm", name=f"rot{s}")
                for m in range(2):
                    nc.tensor.matmul(rot[:, m, :], perm_sb, raws[m],
                                     start=True, stop=True)
                rotk = pmm.tile([128, 2, 512], F32, tag="mm", name=f"rotk{s}")
                nc.tensor.matmul(rotk[0:64, 0, :], perm_sb[0:64, 0:64], rawk,
                                 start=True, stop=True)
                for m in range(2):
                    t1 = wkp.tile([128, 512], F16, tag=f"t1{m}")
                    nc.vector.tensor_tensor(out=t1, in0=rot[:, m, :],
                                            in1=sin_sb[:, sl], op=OP.mult)
                    t2 = wkp.tile([128, 512], F16, tag=f"t2{m}")
                    nc.vector.tensor_tensor(out=t2, in0=raws[m], in1=cos_sb[:, sl],
                                            op=OP.mult)
                    nc.vector.tensor_tensor(out=qT[m][:, sl], in0=t1, in1=t2,
                                            op=OP.add)
                t1k = wkp.tile([64, 512], F16, tag="t1k")
                nc.vector.tensor_tensor(out=t1k, in0=rotk[0:64, 0, :],
                                        in1=sin_sb[0:64, sl], op=OP.mult)
                t2k = wkp.tile([64, 512], F16, tag="t2k")
                nc.vector.tensor_tensor(out=t2k, in0=rawk, in1=cos_sb[0:64, sl],
                                        op=OP.mult)
                nc.vector.tensor_tensor(out=kdup[0:64, sl], in0=t1k, in1=t2k,
                                        op=OP.add)
                nc.gpsimd.dma_start(out=kdup[64:128, sl], in_=kdup[0:64, sl])

            def scores_part(s, part):
                """Scores + exp + band-mask for strip s's k-tiles.

                part 'A': window columns inside strip s (needs only rope(s));
                part 'B': spill-over columns in strip s+1 (needs rope(s+1)).
                The A/B column split at the strip boundary coincides exactly
                with the PV piece boundaries, so q-strip s's PV consumes only
                A-parts of strip s and B-parts of strip s-1 — all available
                one iteration before PV runs."""
                for kt in range(4 * s, 4 * s + 4):
                    j0 = 128 * kt
                    w = min(WIN, S - j0)
                    j = kt % 4
                    c0, c1 = (0, min(w, 512 - 128 * j)) if part == "A" else \
                             (min(w, 512 - 128 * j), w)
                    if c0 >= c1:
                        continue
                    for hp in range(2):
                        if part == "A":
                            pss = pmm.tile([128, 2, 512], F32, tag="mm",
                                           name=f"scA{kt}_{hp}")
                            et = ep.tile([128, 2, 512], F16, tag="e")
                            expm[(kt, hp)] = et
                        else:
                            pss = pmm.tile([128, 2, 512], F32, tag="mm",
                                           name=f"scB{kt}_{hp}")
                            et = expm[(kt, hp)]
                        for hh in range(2):
                            h = 2 * hp + hh
                            m, half = h // 2, (h % 2) * 64
                            nc.tensor.matmul(
                                pss[:, hh, c0:c1],
                                kdup[half:half + 64, bass.ts(kt, 128)],
                                qT[m][half:half + 64, j0 + c0:j0 + c1],
                                start=True, stop=True)
                        nc.scalar.activation(et[:, :, c0:c1], pss[:, :, c0:c1],
                                             AF.Exp, scale=float(SCALE))
                        # per-hh contiguous slices keep the DVE in 2x mode
                        nc.vector.tensor_tensor(out=et[:, 0, c0:c1],
                                                in0=et[:, 0, c0:c1],
                                                in1=mask_sb[:, 0, c0:c1],
                                                op=OP.mult)
                        nc.vector.tensor_tensor(out=et[:, 1, c0:c1],
                                                in0=et[:, 1, c0:c1],
                                                in1=mask_sb[:, 1, c0:c1],
                                                op=OP.mult)

            def pv_norm(s):
                pieces = _pv_pieces(s)
                sl = bass.ts(s, 512)
                pvss = []
                for hp in range(2):
                    pvp = pmm.tile([128, 2, 512], F32, tag="mm", name=f"pv{s}_{hp}")
                    for i, (kt, c0, c1) in enumerate(pieces):
                        base = 128 * kt + c0 - 512 * s
                        for hh in range(2):
                            nc.tensor.matmul(pvp[0:65, hh, base:base + (c1 - c0)],
                                             vaug[:, kt, 0:65],
                                             expm[(kt, hp)][:, hh, c0:c1],
                                             start=(i == 0), stop=(i == len(pieces) - 1),
                                             skip_group_check=True)
                    # evacuate psum immediately; the slot frees in <1us
                    pvs = npool.tile([65, 2, 512], F16, tag=f"pvs{hp}")
                    nc.scalar.copy(pvs, pvp[0:65, :, :])
                    pvss.append(pvs)
                for hp in range(2):
                    pvs = pvss[hp]
                    # denominators (row 64, from the ones column of vaug) are
                    # broadcast to 64 partitions by a K=1 ones matmul, then
                    # reciprocal'd on-chip — no DMA hops on this chain
                    bcp = pmm.tile([128, 2, 512], F32, tag="mm", name=f"bc{s}_{hp}")
                    for hh in range(2):
                        nc.tensor.matmul(bcp[0:64, hh, :], ones64[64:65, :],
                                         pvs[64:65, hh, :], start=True, stop=True)
                    rbc = npool.tile([64, 2, 512], F32, tag=f"rbc{hp}")
                    nc.vector.reciprocal_approx_fast(out=rbc, in_=bcp[0:64, :, :])
                    nc.vector.tensor_tensor(out=aT[hp][0:64, sl],
                                            in0=pvs[0:64, 0, :], in1=rbc[:, 0, :],
                                            op=OP.mult)
                    stag = npool.tile([64, 512], F16, tag="stag")
                    nc.vector.tensor_tensor(out=stag, in0=pvs[0:64, 1, :],
                                            in1=rbc[:, 1, :], op=OP.mult)
                    nc.scalar.dma_start(out=aT[hp][64:128, sl], in_=stag)

            def oproj(s):
                for j in range(4):
                    st = 4 * s + j
                    psos = pmm.tile([128, 2, 512], F32, tag="mm", name=f"o{st}")
                    for ch in range(2):
                        for dsp in range(2):
                            nc.tensor.matmul(psos[:, dsp, :],
                                             aT[ch][:, bass.ts(st, 128)],
                                             wo_sb[:, ch, bass.ts(dsp, 512)],
                                             start=(ch == 0), stop=(ch == 1))
                    ost = opool.tile([128, 1024], F16, tag="o")
                    if j < 2:
                        nc.scalar.copy(ost, psos[:, :, :])
                    else:
                        nc.vector.tensor_copy(ost, psos[:, :, :])
                    nc.sync.dma_start(out=o_part[bass.ts(st, 128), :], in_=ost)

            for i in range(5):
                if i <= 3:
                    proj_strip(i)
                if i >= 1:
                    pv_norm(i - 1)      # inputs complete since iteration i-1
                if i <= 3:
                    scores_part(i, "A")
                if 1 <= i <= 3:
                    scores_part(i - 1, "B")
                if i >= 1:
                    oproj(i - 1)

    nc.compile()
    return nc


_NC = {}


def _get_nc():
    if "k" not in _NC:
        _NC["k"] = _build()
    return _NC["k"]


def _host_inputs(hidden_states, position_ids, Wq, Wk, Wv, Wo):
    hs = np.asarray(hidden_states, np.float32)
    Wq = np.asarray(Wq, np.float32)
    Wk = np.asarray(Wk, np.float32)
    Wv = np.asarray(Wv, np.float32)
    Wo = np.asarray(Wo, np.float32)

    hsT4 = []
    for b in range(B):
        hT = np.ascontiguousarray(hs[b].T).astype(np.float16)      # [D, S]
        # [NST, 128, 8, 512]: strip-major, partition-major within strip
        h4 = np.empty((NST, 128, 8, 512), np.float16)
        for s_ in range(NST):
            for dt in range(8):
                h4[s_, :, dt] = hT[128 * dt:128 * (dt + 1), 512 * s_:512 * (s_ + 1)]
        hsT4.append(np.ascontiguousarray(h4))

    inv_freq = (1.0 / (THETA ** (np.arange(0, HD, 2, dtype=np.float32) / HD))).astype(np.float32)
    cos2, sin2 = [], []
    for b in range(B):
        pos = np.asarray(position_ids[b]).astype(np.float32)
        freqs = pos[:, None] * inv_freq[None, :]          # [S, 32]
        cosf = np.cos(freqs).T                            # [32, S]
        sinf = np.sin(freqs).T
        cos64 = np.concatenate([cosf, cosf], axis=0)      # [64, S]
        sin64s = np.concatenate([-sinf, sinf], axis=0)    # sign-folded
        cos2.append(np.concatenate([cos64, cos64], axis=0).astype(np.float16))
        sin2.append(np.concatenate([sin64s, sin64s], axis=0).astype(np.float16))

    p = np.arange(128)[:, None]
    c = np.arange(WIN)[None, :]
    band = ((p <= c) & (c < p + WINDOW)).astype(np.float16)   # [128, 384]
    mask2 = np.zeros((128, 2, 512), np.float16)
    mask2[:, 0, 0:WIN] = band
    mask2[:, 1, 0:WIN] = band

    perm = np.zeros((64, 64), np.float32)
    for i in range(32):
        perm[i, i + 32] = 1.0
        perm[i + 32, i] = 1.0
    perm2 = np.kron(np.eye(2, dtype=np.float32), perm)    # [128, 128]
    permT = np.ascontiguousarray(perm2.T).astype(np.float16)

    in_maps = []
    for core in range(8):
        b, g = divmod(core, 4)
        # partition-major: [128, n_chunks, cols]
        wqT = np.ascontiguousarray(
            Wq[256 * g:256 * (g + 1)].T.astype(np.float16)
            .reshape(8, 128, 256).transpose(1, 0, 2))
        wkT = np.ascontiguousarray(
            Wk[64 * g:64 * (g + 1)].T.astype(np.float16)
            .reshape(8, 128, 64).transpose(1, 0, 2))
        wvT = np.ascontiguousarray(
            Wv[64 * g:64 * (g + 1)].T.astype(np.float16)
            .reshape(8, 128, 64).transpose(1, 0, 2))
        woT = np.ascontiguousarray(
            Wo[:, 256 * g:256 * (g + 1)].T.astype(np.float16)
            .reshape(2, 128, D).transpose(1, 0, 2))
        in_maps.append({
            "hsT4": hsT4[b], "wqT": wqT, "wkT": wkT, "wvT": wvT, "woT": woT,
            "cos2": cos2[b], "sinS2": sin2[b],
            "bandmask": mask2, "permT": permT,
        })
    return in_maps


def run_spmd(hidden_states, attention_mask, position_ids, Wq, Wk, Wv, Wo, **spmd_kwargs):
    nc = _get_nc()
    in_maps = _host_inputs(hidden_states, position_ids, Wq, Wk, Wv, Wo)
    res = run_bass_kernel_spmd(nc, in_maps, list(range(8)), **spmd_kwargs)
    out = np.zeros((B, S, D), np.float32)
    for core in range(8):
        out[core // 4] += np.asarray(res.results[core]["o_part"], np.float32)
    return out, res


def kernel(hidden_states, attention_mask, position_ids, Wq, Wk, Wv, Wo):
    out, _ = run_spmd(hidden_states, attention_mask, position_ids, Wq, Wk, Wv, Wo)
    return out
